# revision 6
# baseline (speedup 1.0000x reference)
import sys, os, hashlib
sys.path.insert(0, "/opt/trn_rl_repo")
import numpy as np
import ml_dtypes

DIM = 256; DIM_HEAD = 32; HEADS = 8; WSZ = 8; D4 = 64
EPS = 1e-5
SCALE = DIM_HEAD ** -0.5
NCORES = 8
HSH = 64  # H rows per core (one batch quarter)
BF16 = ml_dtypes.bfloat16
NEFF_CACHE_DIR = "/root/.bass_neff_cache"


def _ln_np(x, g, b):
    m = x.mean(-1, keepdims=True)
    v = x.var(-1, keepdims=True)
    return (x - m) / np.sqrt(v + EPS) * g + b


def _dpb_bias64(dpb_w1, dpb_b1, dpb_g1, dpb_beta1,
                dpb_w2, dpb_b2, dpb_g2, dpb_beta2,
                dpb_w3, dpb_b3, dpb_g3, dpb_beta3,
                dpb_w4, dpb_b4):
    pos = np.arange(-WSZ, WSZ + 1, dtype=np.float32)
    rel = np.stack(np.meshgrid(pos, pos, indexing='ij')).reshape(2, -1).T
    h = np.maximum(_ln_np(rel @ dpb_w1.T + dpb_b1, dpb_g1, dpb_beta1), 0)
    h = np.maximum(_ln_np(h @ dpb_w2.T + dpb_b2, dpb_g2, dpb_beta2), 0)
    h = np.maximum(_ln_np(h @ dpb_w3.T + dpb_b3, dpb_g3, dpb_beta3), 0)
    biases = (h @ dpb_w4.T + dpb_b4)[:, 0]
    p = np.arange(WSZ)
    grid = np.stack(np.meshgrid(p, p, indexing='ij')).reshape(2, -1).T
    r = grid[:, None] - grid[None, :] + WSZ - 1
    idx = r[..., 0] * (2 * WSZ - 1) + r[..., 1]
    return biases[idx].astype(np.float32)  # (64, 64)


def build_v2():
    from contextlib import ExitStack
    import concourse.bass as bass
    from concourse import mybir
    from concourse.tile import TileContext

    f32 = mybir.dt.float32
    bf16 = mybir.dt.bfloat16
    AX = mybir.AxisListType.X
    AF = mybir.ActivationFunctionType

    strips = [(hb, ws) for hb in range(8) for ws in range(4)]

    nc = bass.Bass(disable_frame_to_traceback=True)
    x_e = nc.declare_dram_parameter("x", [2, 128, HSH, 256], bf16, isOutput=False)
    wq_e = nc.declare_dram_parameter("wq", [2, 128, 768], bf16, isOutput=False)
    bq_e = nc.declare_dram_parameter("bq", [6, 128, 1], f32, isOutput=False)
    wo_e = nc.declare_dram_parameter("wo", [2, 128, 256], bf16, isOutput=False)
    bo_e = nc.declare_dram_parameter("bo", [2, 128, 1], f32, isOutput=False)
    bm_e = nc.declare_dram_parameter("biasm", [64, 512], f32, isOutput=False)
    idb_e = nc.declare_dram_parameter("idb", [128, 128], bf16, isOutput=False)
    out_e = nc.declare_dram_parameter("out", [2, 128, HSH, 256], bf16, isOutput=True)

    with TileContext(nc) as tc, ExitStack() as ctx:
        cpool = ctx.enter_context(tc.tile_pool(name="consts", bufs=1))
        wq = [cpool.tile([128, 768], bf16, tag=f"wq{i}", name=f"wq{i}") for i in range(2)]
        wo = [cpool.tile([128, 256], bf16, tag=f"wo{i}", name=f"wo{i}") for i in range(2)]
        bq6 = [cpool.tile([128, 1], f32, tag=f"bq{i}", name=f"bq{i}") for i in range(6)]
        bo2 = [cpool.tile([128, 1], f32, tag=f"bo{i}", name=f"bo{i}") for i in range(2)]
        biasm = cpool.tile([64, 512], f32, tag="biasm", name="biasm")
        idb = cpool.tile([128, 128], bf16, tag="idb", name="idb")
        onesb = cpool.tile([128, 128], bf16, tag="onesb", name="onesb")
        nc.vector.memset(onesb[:], 1.0)
        epsb = cpool.tile([128, 1], f32, tag="epsb", name="epsb")
        nc.vector.memset(epsb[:], EPS)
        for i in range(2):
            nc.sync.dma_start(out=wq[i][:], in_=wq_e[i])
            nc.sync.dma_start(out=wo[i][:], in_=wo_e[i])
            nc.sync.dma_start(out=bo2[i][:], in_=bo_e[i])
        for i in range(6):
            nc.sync.dma_start(out=bq6[i][:], in_=bq_e[i])
        nc.sync.dma_start(out=biasm[:], in_=bm_e[:])
        nc.sync.dma_start(out=idb[:], in_=idb_e[:])

        xpool = ctx.enter_context(tc.tile_pool(name="xp", bufs=2))
        spool = ctx.enter_context(tc.tile_pool(name="sp", bufs=2))
        qpool = ctx.enter_context(tc.tile_pool(name="qp", bufs=2))
        apool = ctx.enter_context(tc.tile_pool(name="ap", bufs=2))
        opool = ctx.enter_context(tc.tile_pool(name="op", bufs=2))
        p_st = ctx.enter_context(tc.tile_pool(name="pst", bufs=1, space="PSUM"))
        p_mm = ctx.enter_context(tc.tile_pool(name="pmm", bufs=2, space="PSUM"))
        p_sim = ctx.enter_context(tc.tile_pool(name="psim", bufs=2, space="PSUM"))
        p_tr = ctx.enter_context(tc.tile_pool(name="ptr", bufs=1, space="PSUM"))
        p_av = ctx.enter_context(tc.tile_pool(name="pav", bufs=1, space="PSUM"))

        for (hb, ws) in strips:
            h0, w0 = hb * 8, ws * 64
            # ---- load x rows (contiguous 256B runs per row)
            xt = [xpool.tile([128, 512], bf16, tag=f"xt{c}", name=f"xt{c}") for c in range(2)]
            sq = [xpool.tile([128, 512], bf16, tag=f"sq{c}", name=f"sq{c}") for c in range(2)]
            for c in range(2):
                src = x_e[c, :, h0:h0 + 8, w0:w0 + 64]
                nc.sync.dma_start(out=xt[c][:].rearrange("p (s1 w) -> p s1 w", s1=8), in_=src)
                nc.vector.tensor_mul(sq[c][:], xt[c][:], xt[c][:])
            # ---- channel stats via ones-matmul, replicated across partitions
            sm_ps = p_st.tile([128, 512], f32, tag="sm", name="sm")
            sq_ps = p_st.tile([128, 512], f32, tag="sqs", name="sqs")
            nc.tensor.matmul(sm_ps[:], onesb[:], xt[0][:], start=True, stop=False)
            nc.tensor.matmul(sm_ps[:], onesb[:], xt[1][:], start=False, stop=True)
            nc.tensor.matmul(sq_ps[:], onesb[:], sq[0][:], start=True, stop=False)
            nc.tensor.matmul(sq_ps[:], onesb[:], sq[1][:], start=False, stop=True)
            # ---- LN epilogue (walrus allows only one PSUM input per DVE op)
            ms = spool.tile([128, 512], f32, tag="ms", name="ms")
            nc.scalar.activation(ms[:], sm_ps[:], AF.Copy, scale=1.0 / 256.0)
            t = spool.tile([128, 512], f32, tag="t", name="t")
            nc.scalar.activation(t[:], sq_ps[:], AF.Copy, scale=1.0 / 256.0)
            msq = spool.tile([128, 512], f32, tag="msq", name="msq")
            nc.vector.tensor_mul(msq[:], ms[:], ms[:])
            d = spool.tile([128, 512], f32, tag="d", name="d")
            nc.vector.tensor_sub(d[:], t[:], msq[:])
            sr = spool.tile([128, 512], f32, tag="sr", name="sr")
            nc.scalar.activation(sr[:], d[:], AF.Sqrt, bias=epsb[:])
            rstd = spool.tile([128, 512], f32, tag="rstd", name="rstd")
            nc.vector.reciprocal(rstd[:], sr[:])
            # ---- z = (x - mean) * rstd
            z = [xpool.tile([128, 512], bf16, tag=f"z{c}", name=f"z{c}") for c in range(2)]
            for c in range(2):
                z0 = xpool.tile([128, 512], f32, tag=f"z0{c}", name=f"z0{c}")
                nc.vector.tensor_sub(z0[:], xt[c][:], ms[:])
                nc.vector.tensor_mul(z[c][:], z0[:], rstd[:])
            # ---- QKV projection; evacuate into window-major bf16 tiles.
            # HW erratum found empirically: a matmul operand slice at partition
            # base 32 of a COMPUTED tile wedges the device (DMA-sourced tiles
            # are fine). Store qkv per-head as [32, 512] tiles so every later
            # matmul operand sits at partition base 0.
            # tiles: q_h = h, k_h = 8+h, v_h = 16+h
            qkv = [qpool.tile([32, 512], bf16, tag=f"qkv{e}", name=f"qkv{e}") for e in range(24)]
            for e in range(6):
                ps = p_mm.tile([128, 512], f32, tag="mm", name="mm")
                nc.tensor.matmul(ps[:], wq[0][:, e * 128:(e + 1) * 128], z[0][:], start=True, stop=False)
                nc.tensor.matmul(ps[:], wq[1][:, e * 128:(e + 1) * 128], z[1][:], start=False, stop=True)
                sv = ps[:].rearrange("p (s1 ww s2) -> p s1 ww s2", s1=8, ww=8)
                for l in range(4):
                    dv = qkv[4 * e + l][:].rearrange("p (ww s1 s2) -> p s1 ww s2", ww=8, s1=8)
                    nc.vector.tensor_scalar_add(dv, sv[l * 32:(l + 1) * 32], bq6[e][l * 32:(l + 1) * 32])
            # ---- attention per window (all slices window-contiguous, base 0)
            ao = [apool.tile([128, 512], bf16, tag=f"ao{c}", name=f"ao{c}") for c in range(2)]
            for ww in range(8):
                wc = slice(ww * 64, ww * 64 + 64)
                sim_ps = p_sim.tile([64, 512], f32, tag="sim", name="sim")
                for h in range(HEADS):
                    nc.tensor.matmul(sim_ps[:, h * 64:(h + 1) * 64],
                                     qkv[h][:, wc],
                                     qkv[8 + h][:, wc],
                                     start=True, stop=True)
                at = apool.tile([64, 512], f32, tag="at", name="at")
                nc.vector.tensor_add(at[:], sim_ps[:], biasm[:])
                A = apool.tile([64, 512], bf16, tag="A", name="A")
                nc.scalar.activation(A[:], at[:], AF.Exp)
                dn = spool.tile([64, 8], f32, tag="dn", name="dn")
                nc.vector.reduce_sum(dn[:], A[:].rearrange("p (h j) -> p h j", h=8), axis=AX)
                rc = spool.tile([64, 8], f32, tag="rc", name="rc")
                nc.vector.reciprocal(rc[:], dn[:])
                An = apool.tile([64, 512], bf16, tag="An", name="An")
                nc.vector.tensor_mul(
                    An[:].rearrange("p (h j) -> p h j", h=8),
                    A[:].rearrange("p (h j) -> p h j", h=8),
                    rc[:].unsqueeze(2).broadcast_to([64, 8, 64]))
                # A^T per head-pair transpose [64, 128] -> [128, 64], then split
                # into per-head [64, 64] tiles at partition base 0
                aT = [apool.tile([64, 64], bf16, tag=f"aT{h}", name=f"aT{h}") for h in range(8)]
                for p in range(4):
                    tp = p_tr.tile([128, 64], bf16, tag="trp", name="trp")
                    nc.tensor.transpose(tp[:], An[:, p * 128:(p + 1) * 128], idb[0:64, 0:64])
                    nc.scalar.copy(aT[2 * p][:], tp[0:64, :])
                    nc.scalar.copy(aT[2 * p + 1][:], tp[64:128, :])
                # V^T per head: [32, 64] -> [64, 32], packed [64, 256]
                vT = apool.tile([64, 256], bf16, tag="vT", name="vT")
                for h in range(HEADS):
                    tv = p_tr.tile([64, 64], bf16, tag="trp", name="trp")
                    nc.tensor.transpose(tv[:, 0:32], qkv[16 + h][:, wc], idb[0:32, 0:32])
                    nc.scalar.copy(vT[:, h * 32:(h + 1) * 32], tv[:, 0:32])
                # out2 = V^T A^T, heads packed along columns (no PSUM group overlap)
                av_ps = p_av.tile([32, 512], f32, tag="av", name="av")
                for h in range(HEADS):
                    nc.tensor.matmul(av_ps[:, h * 64:(h + 1) * 64],
                                     vT[:, h * 32:(h + 1) * 32],
                                     aT[h][:],
                                     start=True, stop=True)
                for h in range(HEADS):
                    c, r = h // 4, (h % 4) * 32
                    nc.scalar.copy(ao[c][r:r + 32, wc], av_ps[:, h * 64:(h + 1) * 64])
            # ---- output projection; un-window on evacuation; store
            for c in range(2):
                ps = p_mm.tile([128, 512], f32, tag="mm", name="mm")
                nc.tensor.matmul(ps[:], wo[0][:, c * 128:(c + 1) * 128], ao[0][:], start=True, stop=False)
                nc.tensor.matmul(ps[:], wo[1][:, c * 128:(c + 1) * 128], ao[1][:], start=False, stop=True)
                orm = opool.tile([128, 512], bf16, tag=f"orm{c}", name=f"orm{c}")
                dv = orm[:].rearrange("p (s1 ww s2) -> p ww s1 s2", s1=8, ww=8)
                sv = ps[:].rearrange("p (ww s1 s2) -> p ww s1 s2", ww=8, s1=8)
                nc.vector.tensor_scalar_add(dv, sv, bo2[c][:])
                nc.sync.dma_start(out=out_e[c, :, h0:h0 + 8, w0:w0 + 64],
                                  in_=orm[:].rearrange("p (s1 w) -> p s1 w", s1=8))
    return nc


def _split_multi_waits(nc, max_waits=1):
    # walrus codegen in this container rejects instructions carrying more
    # than one sem-wait ("Too many sync wait commands"). Move excess waits
    # onto InstNoOp carriers inserted just before, on the same engine
    # (engine queues are in-order, so semantics are preserved).
    from concourse import mybir
    n_split = 0
    for fn in nc.m.functions:
        for blk in fn.blocks:
            insts = blk.instructions
            i = 0
            while i < len(insts):
                inst = insts[i]
                si = inst.sync_info
                if si is not None and si.on_wait and len(si.on_wait) > max_waits:
                    waits = list(si.on_wait)
                    keep = waits[-max_waits:]
                    extra = waits[:-max_waits]
                    carriers = []
                    for j in range(0, len(extra), max_waits):
                        chunk = extra[j:j + max_waits]
                        nop = mybir.InstNoOp(
                            name=nc.get_next_instruction_name(),
                            sync_info=mybir.SyncInfo(on_wait=chunk, on_update=[]),
                            bass_nofuse=True,
                            engine=inst.engine,
                        )
                        nc.register_instruction(nop)
                        carriers.append(nop)
                    inst.sync_info = mybir.SyncInfo(
                        on_wait=keep, on_update=list(si.on_update or [])
                    )
                    insts[i:i] = carriers
                    i += len(carriers)
                    n_split += 1
                i += 1
    return n_split


def _install_neff_disk_cache():
    # cache walrus-compiled NEFFs on disk keyed by BIR bytes, so repeat runs
    # (including fresh processes) skip the multi-minute backend compile
    import concourse.bass2jax as b2j
    if getattr(b2j, "_neff_cache_installed", False):
        return
    orig = b2j.compile_bir_kernel

    def cached(bir_json, tmpdir, neff_name="file.neff"):
        try:
            os.makedirs(NEFF_CACHE_DIR, exist_ok=True)
            key = hashlib.sha256(bir_json).hexdigest()
            path = os.path.join(NEFF_CACHE_DIR, key + ".neff")
            if os.path.exists(path):
                dst = os.path.join(tmpdir, neff_name)
                with open(path, "rb") as f, open(dst, "wb") as g:
                    g.write(f.read())
                return dst
            out = orig(bir_json, tmpdir, neff_name)
            with open(out, "rb") as f:
                data = f.read()
            tmp = path + ".tmp"
            with open(tmp, "wb") as f:
                f.write(data)
            os.replace(tmp, path)
            return out
        except Exception:
            return orig(bir_json, tmpdir, neff_name)

    b2j.compile_bir_kernel = cached
    b2j._neff_cache_installed = True


def prep_consts(norm_g, norm_b, w_qkv, w_out, b_out, **dpb):
    g = np.asarray(norm_g, np.float32).reshape(DIM)
    bvec = np.asarray(norm_b, np.float32).reshape(DIM)
    W = np.asarray(w_qkv, np.float32)
    Wg = W * g[None, :]
    Wg[:256] *= SCALE
    bq = W @ bvec
    bq = bq.copy(); bq[:256] *= SCALE
    wq = np.ascontiguousarray(Wg.T.reshape(2, 128, 768)).astype(BF16)
    bq6 = np.ascontiguousarray(bq.reshape(6, 128, 1)).astype(np.float32)
    wo = np.ascontiguousarray(np.asarray(w_out, np.float32).T.reshape(2, 128, 256)).astype(BF16)
    bo = np.ascontiguousarray(np.asarray(b_out, np.float32).reshape(2, 128, 1))
    bias64 = _dpb_bias64(**{k: np.asarray(v, np.float32) for k, v in dpb.items()})
    biasm = np.ascontiguousarray(np.tile(bias64, (1, 8)))
    idb = np.eye(128).astype(BF16)
    return dict(wq=wq, bq=bq6, wo=wo, bo=bo, biasm=biasm, idb=idb)


LAST = None

# order matches build_v2's declare_dram_parameter calls (x first, out excluded)
IN_NAMES = ["x", "wq", "bq", "wo", "bo", "biasm", "idb"]
EXPORT_VERSION = "v3"


def _export_cache_path():
    import inspect
    key = hashlib.sha256(
        (inspect.getsource(build_v2) + EXPORT_VERSION).encode()).hexdigest()[:24]
    return os.path.join(NEFF_CACHE_DIR, f"export_{key}.bin")


def _patch_bass_effect():
    import concourse.bass2jax as b2j
    # jax.export requires effects to be reconstructible via a nullary
    # constructor producing an equal object; BassEffect is stateless
    b2j.BassEffect.__eq__ = lambda self, other: isinstance(other, b2j.BassEffect)
    b2j.BassEffect.__hash__ = lambda self: hash(b2j.BassEffect)


def _make_exported():
    # build the bass module and export the lowered sharded call (BIR is
    # embedded in the custom-call backend_config, so the deserialized module
    # no longer needs bass at all; output zeros are created on-device inside)
    import jax
    import jax.export
    import jax.numpy as jnp
    from jax.experimental.shard_map import shard_map
    from jax.sharding import Mesh, PartitionSpec
    from concourse import mybir
    import concourse.bass2jax as b2j

    nc = build_v2()
    _split_multi_waits(nc)
    b2j.install_neuronx_cc_hook()
    _patch_bass_effect()

    partition_name = nc.partition_id_tensor.name if nc.partition_id_tensor else None
    in_names, out_names, out_avals = [], [], []
    for alloc in nc.m.functions[0].allocations:
        if not isinstance(alloc, mybir.MemoryLocationSet):
            continue
        name = alloc.memorylocations[0].name
        if alloc.kind == "ExternalInput":
            if name != partition_name:
                in_names.append(name)
        elif alloc.kind == "ExternalOutput":
            out_names.append(name)
            out_avals.append(jax.core.ShapedArray(tuple(alloc.tensor_shape),
                                                  mybir.dt.np(alloc.dtype)))
    assert in_names == IN_NAMES, in_names
    all_names = list(in_names) + list(out_names)
    if partition_name is not None:
        all_names.append(partition_name)

    def _body(*args):
        operands = list(args)
        for a in out_avals:
            operands.append(jnp.zeros(a.shape, a.dtype))
        if partition_name is not None:
            operands.append(b2j.partition_id_tensor())
        outs = b2j._bass_exec_p.bind(
            *operands,
            out_avals=tuple(out_avals),
            in_names=tuple(all_names),
            out_names=tuple(out_names),
            lowering_input_output_aliases=(),
            sim_require_finite=True,
            sim_require_nnan=True,
            nc=nc,
        )
        return tuple(outs)

    mesh = Mesh(np.asarray(jax.devices()[:NCORES]), ("core",))
    sharded = jax.jit(
        shard_map(_body, mesh=mesh,
                  in_specs=(PartitionSpec("core"),) * len(in_names),
                  out_specs=(PartitionSpec("core"),) * len(out_names),
                  check_rep=False))
    shapes = {"x": (2, 128, HSH, 256), "wq": (2, 128, 768), "bq": (6, 128, 1),
              "wo": (2, 128, 256), "bo": (2, 128, 1), "biasm": (64, 512),
              "idb": (128, 128)}
    dts = {"x": BF16, "wq": BF16, "bq": np.float32, "wo": BF16,
           "bo": np.float32, "biasm": np.float32, "idb": BF16}
    args = [jax.ShapeDtypeStruct((NCORES * shapes[nm][0], *shapes[nm][1:]), dts[nm])
            for nm in in_names]
    dsc = jax.export.DisabledSafetyCheck.custom_call("bass_exec")
    return jax.export.export(sharded, disabled_checks=[dsc])(*args)


def _get_exported():
    import jax.export
    _install_neff_disk_cache()
    _patch_bass_effect()
    path = _export_cache_path()
    if os.path.exists(path):
        try:
            return jax.export.deserialize(open(path, "rb").read())
        except Exception:
            pass
    exp = _make_exported()
    try:
        os.makedirs(NEFF_CACHE_DIR, exist_ok=True)
        tmp = path + ".tmp"
        with open(tmp, "wb") as f:
            f.write(exp.serialize())
        os.replace(tmp, path)
    except Exception:
        pass
    return exp


def _kernel_bass(x, consts):
    global LAST
    import jax
    exp = _get_exported()
    import concourse.bass2jax as b2j
    b2j.install_neuronx_cc_hook()

    xs = np.concatenate([
        np.ascontiguousarray(x[i // 4, :, (i % 4) * 64:(i % 4) * 64 + 64, :])
        .reshape(2, 128, HSH, 256).astype(BF16)
        for i in range(NCORES)], axis=0)
    gin = [xs] + [np.concatenate([consts[nm]] * NCORES, axis=0)
                  for nm in IN_NAMES[1:]]
    out = jax.jit(exp.call)(*gin)
    out0 = np.asarray(out[0] if isinstance(out, (tuple, list)) else out)
    res = out0.reshape(NCORES, 2, 128, HSH, 256)
    full = np.empty((2, DIM, 256, 256), dtype=np.float32)
    for i in range(NCORES):
        b, r0 = i // 4, (i % 4) * 64
        full[b, :, r0:r0 + 64, :] = res[i].reshape(256, 64, 256).astype(np.float32)
    return full


def _kernel_numpy(x, norm_g, norm_b, w_qkv, w_out, b_out, **dpb):
    # fallback: straight port of the reference in numpy (f32)
    B, D, H, W = x.shape
    nh, nw = H // WSZ, W // WSZ
    mean = x.mean(axis=1, keepdims=True)
    var = x.var(axis=1, keepdims=True)
    xn = (x - mean) / np.sqrt(var + EPS) * norm_g + norm_b
    xw = xn.reshape(B, D, nh, WSZ, nw, WSZ).transpose(0, 2, 4, 1, 3, 5)
    xw = xw.reshape(B * nh * nw, D, WSZ * WSZ)
    qkv = np.einsum('ed,bdn->ben', w_qkv, xw)
    q, k, v = np.split(qkv, 3, axis=1)
    th = lambda t: t.reshape(-1, HEADS, DIM_HEAD, WSZ * WSZ).transpose(0, 1, 3, 2)
    q, k, v = th(q) * SCALE, th(k), th(v)
    sim = np.einsum('bhid,bhjd->bhij', q, k)
    sim = sim + _dpb_bias64(**dpb)[None, None]
    sim = sim - sim.max(-1, keepdims=True)
    e = np.exp(sim)
    attn = e / e.sum(-1, keepdims=True)
    o = np.einsum('bhij,bhjd->bhid', attn, v)
    o = o.transpose(0, 1, 3, 2).reshape(-1, HEADS * DIM_HEAD, WSZ * WSZ)
    o = np.einsum('de,ben->bdn', w_out, o) + b_out[None, :, None]
    o = o.reshape(B, nh, nw, D, WSZ, WSZ).transpose(0, 3, 1, 4, 2, 5).reshape(B, D, H, W)
    return o.astype(np.float32)


def kernel(**inputs):
    inputs = {k: np.asarray(v) for k, v in inputs.items()}
    try:
        consts = prep_consts(**{k: v for k, v in inputs.items() if k != "x"})
        return _kernel_bass(np.asarray(inputs["x"], np.float32), consts)
    except Exception as ex:
        sys.stderr.write(f"[kernel] bass path failed ({ex!r}); numpy fallback\n")
        return _kernel_numpy(**inputs)


# revision 7
# speedup vs baseline: 1.7621x; 1.7621x over previous
import sys, os, hashlib
sys.path.insert(0, "/opt/trn_rl_repo")
import numpy as np
import ml_dtypes

DIM = 256; DIM_HEAD = 32; HEADS = 8; WSZ = 8; D4 = 64
EPS = 1e-5
SCALE = DIM_HEAD ** -0.5
NCORES = 8
HSH = 64  # H rows per core (one batch quarter)
BF16 = ml_dtypes.bfloat16
NEFF_CACHE_DIR = "/root/.bass_neff_cache"


def _ln_np(x, g, b):
    m = x.mean(-1, keepdims=True)
    v = x.var(-1, keepdims=True)
    return (x - m) / np.sqrt(v + EPS) * g + b


def _dpb_bias64(dpb_w1, dpb_b1, dpb_g1, dpb_beta1,
                dpb_w2, dpb_b2, dpb_g2, dpb_beta2,
                dpb_w3, dpb_b3, dpb_g3, dpb_beta3,
                dpb_w4, dpb_b4):
    pos = np.arange(-WSZ, WSZ + 1, dtype=np.float32)
    rel = np.stack(np.meshgrid(pos, pos, indexing='ij')).reshape(2, -1).T
    h = np.maximum(_ln_np(rel @ dpb_w1.T + dpb_b1, dpb_g1, dpb_beta1), 0)
    h = np.maximum(_ln_np(h @ dpb_w2.T + dpb_b2, dpb_g2, dpb_beta2), 0)
    h = np.maximum(_ln_np(h @ dpb_w3.T + dpb_b3, dpb_g3, dpb_beta3), 0)
    biases = (h @ dpb_w4.T + dpb_b4)[:, 0]
    p = np.arange(WSZ)
    grid = np.stack(np.meshgrid(p, p, indexing='ij')).reshape(2, -1).T
    r = grid[:, None] - grid[None, :] + WSZ - 1
    idx = r[..., 0] * (2 * WSZ - 1) + r[..., 1]
    return biases[idx].astype(np.float32)  # (64, 64)


def build_v2():
    from contextlib import ExitStack
    import concourse.bass as bass
    from concourse import mybir
    from concourse.tile import TileContext

    f32 = mybir.dt.float32
    bf16 = mybir.dt.bfloat16
    AX = mybir.AxisListType.X
    AF = mybir.ActivationFunctionType

    strips = [(hb, ws) for hb in range(8) for ws in range(4)]

    nc = bass.Bass(disable_frame_to_traceback=True)
    x_e = nc.declare_dram_parameter("x", [2, 128, HSH, 256], bf16, isOutput=False)
    wq_e = nc.declare_dram_parameter("wq", [2, 128, 768], bf16, isOutput=False)
    bq_e = nc.declare_dram_parameter("bq", [6, 128, 1], f32, isOutput=False)
    wo_e = nc.declare_dram_parameter("wo", [2, 128, 256], bf16, isOutput=False)
    bo_e = nc.declare_dram_parameter("bo", [2, 128, 1], f32, isOutput=False)
    bm_e = nc.declare_dram_parameter("biasm", [64, 512], f32, isOutput=False)
    idb_e = nc.declare_dram_parameter("idb", [128, 128], bf16, isOutput=False)
    out_e = nc.declare_dram_parameter("out", [2, 128, HSH, 256], bf16, isOutput=True)

    with TileContext(nc) as tc, ExitStack() as ctx:
        cpool = ctx.enter_context(tc.tile_pool(name="consts", bufs=1))
        wq = [cpool.tile([128, 768], bf16, tag=f"wq{i}", name=f"wq{i}") for i in range(2)]
        wo = [cpool.tile([128, 256], bf16, tag=f"wo{i}", name=f"wo{i}") for i in range(2)]
        bq6 = [cpool.tile([128, 1], f32, tag=f"bq{i}", name=f"bq{i}") for i in range(6)]
        bo2 = [cpool.tile([128, 1], f32, tag=f"bo{i}", name=f"bo{i}") for i in range(2)]
        biasm = cpool.tile([64, 512], f32, tag="biasm", name="biasm")
        idb = cpool.tile([128, 128], bf16, tag="idb", name="idb")
        onesb = cpool.tile([128, 128], bf16, tag="onesb", name="onesb")
        nc.vector.memset(onesb[:], 1.0)
        epsb = cpool.tile([128, 1], f32, tag="epsb", name="epsb")
        nc.vector.memset(epsb[:], EPS)
        for i in range(2):
            nc.sync.dma_start(out=wq[i][:], in_=wq_e[i])
            nc.sync.dma_start(out=wo[i][:], in_=wo_e[i])
            nc.sync.dma_start(out=bo2[i][:], in_=bo_e[i])
        for i in range(6):
            nc.sync.dma_start(out=bq6[i][:], in_=bq_e[i])
        nc.sync.dma_start(out=biasm[:], in_=bm_e[:])
        nc.sync.dma_start(out=idb[:], in_=idb_e[:])

        xpool = ctx.enter_context(tc.tile_pool(name="xp", bufs=2))
        spool = ctx.enter_context(tc.tile_pool(name="sp", bufs=2))
        qpool = ctx.enter_context(tc.tile_pool(name="qp", bufs=2))
        apool = ctx.enter_context(tc.tile_pool(name="ap", bufs=2))
        opool = ctx.enter_context(tc.tile_pool(name="op", bufs=2))
        p_st = ctx.enter_context(tc.tile_pool(name="pst", bufs=1, space="PSUM"))
        p_mm = ctx.enter_context(tc.tile_pool(name="pmm", bufs=2, space="PSUM"))
        p_sim = ctx.enter_context(tc.tile_pool(name="psim", bufs=2, space="PSUM"))
        p_tr = ctx.enter_context(tc.tile_pool(name="ptr", bufs=1, space="PSUM"))
        p_av = ctx.enter_context(tc.tile_pool(name="pav", bufs=1, space="PSUM"))

        for (hb, ws) in strips:
            h0, w0 = hb * 8, ws * 64
            # ---- load x rows (contiguous 256B runs per row)
            xt = [xpool.tile([128, 512], bf16, tag=f"xt{c}", name=f"xt{c}") for c in range(2)]
            sq = [xpool.tile([128, 512], bf16, tag=f"sq{c}", name=f"sq{c}") for c in range(2)]
            for c in range(2):
                src = x_e[c, :, h0:h0 + 8, w0:w0 + 64]
                nc.sync.dma_start(out=xt[c][:].rearrange("p (s1 w) -> p s1 w", s1=8), in_=src)
                nc.vector.tensor_mul(sq[c][:], xt[c][:], xt[c][:])
            # ---- channel stats via ones-matmul, replicated across partitions
            sm_ps = p_st.tile([128, 512], f32, tag="sm", name="sm")
            sq_ps = p_st.tile([128, 512], f32, tag="sqs", name="sqs")
            nc.tensor.matmul(sm_ps[:], onesb[:], xt[0][:], start=True, stop=False)
            nc.tensor.matmul(sm_ps[:], onesb[:], xt[1][:], start=False, stop=True)
            nc.tensor.matmul(sq_ps[:], onesb[:], sq[0][:], start=True, stop=False)
            nc.tensor.matmul(sq_ps[:], onesb[:], sq[1][:], start=False, stop=True)
            # ---- LN epilogue (walrus allows only one PSUM input per DVE op)
            ms = spool.tile([128, 512], f32, tag="ms", name="ms")
            nc.scalar.activation(ms[:], sm_ps[:], AF.Copy, scale=1.0 / 256.0)
            t = spool.tile([128, 512], f32, tag="t", name="t")
            nc.scalar.activation(t[:], sq_ps[:], AF.Copy, scale=1.0 / 256.0)
            msq = spool.tile([128, 512], f32, tag="msq", name="msq")
            nc.vector.tensor_mul(msq[:], ms[:], ms[:])
            d = spool.tile([128, 512], f32, tag="d", name="d")
            nc.vector.tensor_sub(d[:], t[:], msq[:])
            sr = spool.tile([128, 512], f32, tag="sr", name="sr")
            nc.scalar.activation(sr[:], d[:], AF.Sqrt, bias=epsb[:])
            rstd = spool.tile([128, 512], f32, tag="rstd", name="rstd")
            nc.vector.reciprocal(rstd[:], sr[:])
            # ---- z = (x - mean) * rstd
            z = [xpool.tile([128, 512], bf16, tag=f"z{c}", name=f"z{c}") for c in range(2)]
            for c in range(2):
                z0 = xpool.tile([128, 512], f32, tag=f"z0{c}", name=f"z0{c}")
                nc.vector.tensor_sub(z0[:], xt[c][:], ms[:])
                nc.vector.tensor_mul(z[c][:], z0[:], rstd[:])
            # ---- QKV projection; evacuate into window-major bf16 tiles.
            # HW erratum found empirically: a matmul operand slice at partition
            # base 32 of a COMPUTED tile wedges the device (DMA-sourced tiles
            # are fine). Store qkv per-head as [32, 512] tiles so every later
            # matmul operand sits at partition base 0.
            # tiles: q_h = h, k_h = 8+h, v_h = 16+h
            qkv = [qpool.tile([32, 512], bf16, tag=f"qkv{e}", name=f"qkv{e}") for e in range(24)]
            for e in range(6):
                ps = p_mm.tile([128, 512], f32, tag="mm", name="mm")
                nc.tensor.matmul(ps[:], wq[0][:, e * 128:(e + 1) * 128], z[0][:], start=True, stop=False)
                nc.tensor.matmul(ps[:], wq[1][:, e * 128:(e + 1) * 128], z[1][:], start=False, stop=True)
                sv = ps[:].rearrange("p (s1 ww s2) -> p s1 ww s2", s1=8, ww=8)
                for l in range(4):
                    dv = qkv[4 * e + l][:].rearrange("p (ww s1 s2) -> p s1 ww s2", ww=8, s1=8)
                    nc.vector.tensor_scalar_add(dv, sv[l * 32:(l + 1) * 32], bq6[e][l * 32:(l + 1) * 32])
            # ---- attention per window (all slices window-contiguous, base 0)
            ao = [apool.tile([128, 512], bf16, tag=f"ao{c}", name=f"ao{c}") for c in range(2)]
            for ww in range(8):
                wc = slice(ww * 64, ww * 64 + 64)
                sim_ps = p_sim.tile([64, 512], f32, tag="sim", name="sim")
                for h in range(HEADS):
                    nc.tensor.matmul(sim_ps[:, h * 64:(h + 1) * 64],
                                     qkv[h][:, wc],
                                     qkv[8 + h][:, wc],
                                     start=True, stop=True)
                at = apool.tile([64, 512], f32, tag="at", name="at")
                nc.vector.tensor_add(at[:], sim_ps[:], biasm[:])
                A = apool.tile([64, 512], bf16, tag="A", name="A")
                nc.scalar.activation(A[:], at[:], AF.Exp)
                dn = spool.tile([64, 8], f32, tag="dn", name="dn")
                nc.vector.reduce_sum(dn[:], A[:].rearrange("p (h j) -> p h j", h=8), axis=AX)
                rc = spool.tile([64, 8], f32, tag="rc", name="rc")
                nc.vector.reciprocal(rc[:], dn[:])
                An = apool.tile([64, 512], bf16, tag="An", name="An")
                nc.vector.tensor_mul(
                    An[:].rearrange("p (h j) -> p h j", h=8),
                    A[:].rearrange("p (h j) -> p h j", h=8),
                    rc[:].unsqueeze(2).broadcast_to([64, 8, 64]))
                # A^T per head-pair transpose [64, 128] -> [128, 64], then split
                # into per-head [64, 64] tiles at partition base 0
                aT = [apool.tile([64, 64], bf16, tag=f"aT{h}", name=f"aT{h}") for h in range(8)]
                for p in range(4):
                    tp = p_tr.tile([128, 64], bf16, tag="trp", name="trp")
                    nc.tensor.transpose(tp[:], An[:, p * 128:(p + 1) * 128], idb[0:64, 0:64])
                    nc.scalar.copy(aT[2 * p][:], tp[0:64, :])
                    nc.scalar.copy(aT[2 * p + 1][:], tp[64:128, :])
                # V^T per head: [32, 64] -> [64, 32], packed [64, 256]
                vT = apool.tile([64, 256], bf16, tag="vT", name="vT")
                for h in range(HEADS):
                    tv = p_tr.tile([64, 64], bf16, tag="trp", name="trp")
                    nc.tensor.transpose(tv[:, 0:32], qkv[16 + h][:, wc], idb[0:32, 0:32])
                    nc.scalar.copy(vT[:, h * 32:(h + 1) * 32], tv[:, 0:32])
                # out2 = V^T A^T, heads packed along columns (no PSUM group overlap)
                av_ps = p_av.tile([32, 512], f32, tag="av", name="av")
                for h in range(HEADS):
                    nc.tensor.matmul(av_ps[:, h * 64:(h + 1) * 64],
                                     vT[:, h * 32:(h + 1) * 32],
                                     aT[h][:],
                                     start=True, stop=True)
                for h in range(HEADS):
                    c, r = h // 4, (h % 4) * 32
                    nc.scalar.copy(ao[c][r:r + 32, wc], av_ps[:, h * 64:(h + 1) * 64])
            # ---- output projection; un-window on evacuation; store
            for c in range(2):
                ps = p_mm.tile([128, 512], f32, tag="mm", name="mm")
                nc.tensor.matmul(ps[:], wo[0][:, c * 128:(c + 1) * 128], ao[0][:], start=True, stop=False)
                nc.tensor.matmul(ps[:], wo[1][:, c * 128:(c + 1) * 128], ao[1][:], start=False, stop=True)
                orm = opool.tile([128, 512], bf16, tag=f"orm{c}", name=f"orm{c}")
                dv = orm[:].rearrange("p (s1 ww s2) -> p ww s1 s2", s1=8, ww=8)
                sv = ps[:].rearrange("p (ww s1 s2) -> p ww s1 s2", ww=8, s1=8)
                nc.vector.tensor_scalar_add(dv, sv, bo2[c][:])
                nc.sync.dma_start(out=out_e[c, :, h0:h0 + 8, w0:w0 + 64],
                                  in_=orm[:].rearrange("p (s1 w) -> p s1 w", s1=8))
    return nc


def _split_multi_waits(nc, max_waits=1):
    # walrus codegen in this container rejects instructions carrying more
    # than one sem-wait ("Too many sync wait commands"). Move excess waits
    # onto InstNoOp carriers inserted just before, on the same engine
    # (engine queues are in-order, so semantics are preserved).
    from concourse import mybir
    n_split = 0
    for fn in nc.m.functions:
        for blk in fn.blocks:
            insts = blk.instructions
            i = 0
            while i < len(insts):
                inst = insts[i]
                si = inst.sync_info
                if si is not None and si.on_wait and len(si.on_wait) > max_waits:
                    waits = list(si.on_wait)
                    keep = waits[-max_waits:]
                    extra = waits[:-max_waits]
                    carriers = []
                    for j in range(0, len(extra), max_waits):
                        chunk = extra[j:j + max_waits]
                        nop = mybir.InstNoOp(
                            name=nc.get_next_instruction_name(),
                            sync_info=mybir.SyncInfo(on_wait=chunk, on_update=[]),
                            bass_nofuse=True,
                            engine=inst.engine,
                        )
                        nc.register_instruction(nop)
                        carriers.append(nop)
                    inst.sync_info = mybir.SyncInfo(
                        on_wait=keep, on_update=list(si.on_update or [])
                    )
                    insts[i:i] = carriers
                    i += len(carriers)
                    n_split += 1
                i += 1
    return n_split


def _install_neff_disk_cache():
    # cache walrus-compiled NEFFs on disk keyed by BIR bytes, so repeat runs
    # (including fresh processes) skip the multi-minute backend compile
    import concourse.bass2jax as b2j
    if getattr(b2j, "_neff_cache_installed", False):
        return
    orig = b2j.compile_bir_kernel

    def cached(bir_json, tmpdir, neff_name="file.neff"):
        try:
            os.makedirs(NEFF_CACHE_DIR, exist_ok=True)
            key = hashlib.sha256(bir_json).hexdigest()
            path = os.path.join(NEFF_CACHE_DIR, key + ".neff")
            if os.path.exists(path):
                dst = os.path.join(tmpdir, neff_name)
                with open(path, "rb") as f, open(dst, "wb") as g:
                    g.write(f.read())
                return dst
            out = orig(bir_json, tmpdir, neff_name)
            with open(out, "rb") as f:
                data = f.read()
            tmp = path + ".tmp"
            with open(tmp, "wb") as f:
                f.write(data)
            os.replace(tmp, path)
            return out
        except Exception:
            return orig(bir_json, tmpdir, neff_name)

    b2j.compile_bir_kernel = cached
    b2j._neff_cache_installed = True


def prep_consts(norm_g, norm_b, w_qkv, w_out, b_out, **dpb):
    g = np.asarray(norm_g, np.float32).reshape(DIM)
    bvec = np.asarray(norm_b, np.float32).reshape(DIM)
    W = np.asarray(w_qkv, np.float32)
    Wg = W * g[None, :]
    Wg[:256] *= SCALE
    bq = W @ bvec
    bq = bq.copy(); bq[:256] *= SCALE
    wq = np.ascontiguousarray(Wg.T.reshape(2, 128, 768)).astype(BF16)
    bq6 = np.ascontiguousarray(bq.reshape(6, 128, 1)).astype(np.float32)
    wo = np.ascontiguousarray(np.asarray(w_out, np.float32).T.reshape(2, 128, 256)).astype(BF16)
    bo = np.ascontiguousarray(np.asarray(b_out, np.float32).reshape(2, 128, 1))
    bias64 = _dpb_bias64(**{k: np.asarray(v, np.float32) for k, v in dpb.items()})
    biasm = np.ascontiguousarray(np.tile(bias64, (1, 8)))
    idb = np.eye(128).astype(BF16)
    return dict(wq=wq, bq=bq6, wo=wo, bo=bo, biasm=biasm, idb=idb)


LAST = None

# order matches build_v2's declare_dram_parameter calls (x first, out excluded)
IN_NAMES = ["x", "wq", "bq", "wo", "bo", "biasm", "idb"]
EXPORT_VERSION = "v3"


def _export_cache_path():
    import inspect
    key = hashlib.sha256(
        (inspect.getsource(build_v2) + EXPORT_VERSION).encode()).hexdigest()[:24]
    return os.path.join(NEFF_CACHE_DIR, f"export_{key}.bin")


def _patch_bass_effect():
    import concourse.bass2jax as b2j
    # jax.export requires effects to be reconstructible via a nullary
    # constructor producing an equal object; BassEffect is stateless
    b2j.BassEffect.__eq__ = lambda self, other: isinstance(other, b2j.BassEffect)
    b2j.BassEffect.__hash__ = lambda self: hash(b2j.BassEffect)


def _make_exported():
    # build the bass module and export the lowered sharded call (BIR is
    # embedded in the custom-call backend_config, so the deserialized module
    # no longer needs bass at all; output zeros are created on-device inside)
    import jax
    import jax.export
    import jax.numpy as jnp
    from jax.experimental.shard_map import shard_map
    from jax.sharding import Mesh, PartitionSpec
    from concourse import mybir
    import concourse.bass2jax as b2j

    nc = build_v2()
    _split_multi_waits(nc)
    b2j.install_neuronx_cc_hook()
    _patch_bass_effect()

    partition_name = nc.partition_id_tensor.name if nc.partition_id_tensor else None
    in_names, out_names, out_avals = [], [], []
    for alloc in nc.m.functions[0].allocations:
        if not isinstance(alloc, mybir.MemoryLocationSet):
            continue
        name = alloc.memorylocations[0].name
        if alloc.kind == "ExternalInput":
            if name != partition_name:
                in_names.append(name)
        elif alloc.kind == "ExternalOutput":
            out_names.append(name)
            out_avals.append(jax.core.ShapedArray(tuple(alloc.tensor_shape),
                                                  mybir.dt.np(alloc.dtype)))
    assert in_names == IN_NAMES, in_names
    all_names = list(in_names) + list(out_names)
    if partition_name is not None:
        all_names.append(partition_name)

    def _body(*args):
        operands = list(args)
        for a in out_avals:
            operands.append(jnp.zeros(a.shape, a.dtype))
        if partition_name is not None:
            operands.append(b2j.partition_id_tensor())
        outs = b2j._bass_exec_p.bind(
            *operands,
            out_avals=tuple(out_avals),
            in_names=tuple(all_names),
            out_names=tuple(out_names),
            lowering_input_output_aliases=(),
            sim_require_finite=True,
            sim_require_nnan=True,
            nc=nc,
        )
        return tuple(outs)

    mesh = Mesh(np.asarray(jax.devices()[:NCORES]), ("core",))
    sharded = jax.jit(
        shard_map(_body, mesh=mesh,
                  in_specs=(PartitionSpec("core"),) * len(in_names),
                  out_specs=(PartitionSpec("core"),) * len(out_names),
                  check_rep=False))
    shapes = {"x": (2, 128, HSH, 256), "wq": (2, 128, 768), "bq": (6, 128, 1),
              "wo": (2, 128, 256), "bo": (2, 128, 1), "biasm": (64, 512),
              "idb": (128, 128)}
    dts = {"x": BF16, "wq": BF16, "bq": np.float32, "wo": BF16,
           "bo": np.float32, "biasm": np.float32, "idb": BF16}
    args = [jax.ShapeDtypeStruct((NCORES * shapes[nm][0], *shapes[nm][1:]), dts[nm])
            for nm in in_names]
    dsc = jax.export.DisabledSafetyCheck.custom_call("bass_exec")
    return jax.export.export(sharded, disabled_checks=[dsc])(*args)


def _get_exported():
    import jax.export
    _install_neff_disk_cache()
    _patch_bass_effect()
    path = _export_cache_path()
    if os.path.exists(path):
        try:
            return jax.export.deserialize(open(path, "rb").read())
        except Exception:
            pass
    exp = _make_exported()
    try:
        os.makedirs(NEFF_CACHE_DIR, exist_ok=True)
        tmp = path + ".tmp"
        with open(tmp, "wb") as f:
            f.write(exp.serialize())
        os.replace(tmp, path)
    except Exception:
        pass
    return exp


def _kernel_bass(x, consts):
    global LAST
    import jax
    exp = _get_exported()
    import concourse.bass2jax as b2j
    b2j.install_neuronx_cc_hook()

    xs = np.concatenate([
        np.ascontiguousarray(x[i // 4, :, (i % 4) * 64:(i % 4) * 64 + 64, :])
        .reshape(2, 128, HSH, 256).astype(BF16)
        for i in range(NCORES)], axis=0)
    gin = [xs] + [np.concatenate([consts[nm]] * NCORES, axis=0)
                  for nm in IN_NAMES[1:]]
    from jax.sharding import Mesh, PartitionSpec, NamedSharding
    mesh = Mesh(np.asarray(jax.devices()[:NCORES]), ("core",))
    sh = NamedSharding(mesh, PartitionSpec("core"))
    out = jax.jit(exp.call, in_shardings=(sh,) * len(gin),
                  out_shardings=sh)(*gin)
    out0 = np.asarray(out[0] if isinstance(out, (tuple, list)) else out)
    res = out0.reshape(NCORES, 2, 128, HSH, 256)
    full = np.empty((2, DIM, 256, 256), dtype=np.float32)
    for i in range(NCORES):
        b, r0 = i // 4, (i % 4) * 64
        full[b, :, r0:r0 + 64, :] = res[i].reshape(256, 64, 256).astype(np.float32)
    return full


def _kernel_numpy(x, norm_g, norm_b, w_qkv, w_out, b_out, **dpb):
    # fallback: straight port of the reference in numpy (f32)
    B, D, H, W = x.shape
    nh, nw = H // WSZ, W // WSZ
    mean = x.mean(axis=1, keepdims=True)
    var = x.var(axis=1, keepdims=True)
    xn = (x - mean) / np.sqrt(var + EPS) * norm_g + norm_b
    xw = xn.reshape(B, D, nh, WSZ, nw, WSZ).transpose(0, 2, 4, 1, 3, 5)
    xw = xw.reshape(B * nh * nw, D, WSZ * WSZ)
    qkv = np.einsum('ed,bdn->ben', w_qkv, xw)
    q, k, v = np.split(qkv, 3, axis=1)
    th = lambda t: t.reshape(-1, HEADS, DIM_HEAD, WSZ * WSZ).transpose(0, 1, 3, 2)
    q, k, v = th(q) * SCALE, th(k), th(v)
    sim = np.einsum('bhid,bhjd->bhij', q, k)
    sim = sim + _dpb_bias64(**dpb)[None, None]
    sim = sim - sim.max(-1, keepdims=True)
    e = np.exp(sim)
    attn = e / e.sum(-1, keepdims=True)
    o = np.einsum('bhij,bhjd->bhid', attn, v)
    o = o.transpose(0, 1, 3, 2).reshape(-1, HEADS * DIM_HEAD, WSZ * WSZ)
    o = np.einsum('de,ben->bdn', w_out, o) + b_out[None, :, None]
    o = o.reshape(B, nh, nw, D, WSZ, WSZ).transpose(0, 3, 1, 4, 2, 5).reshape(B, D, H, W)
    return o.astype(np.float32)


def kernel(**inputs):
    inputs = {k: np.asarray(v) for k, v in inputs.items()}
    try:
        consts = prep_consts(**{k: v for k, v in inputs.items() if k != "x"})
        return _kernel_bass(np.asarray(inputs["x"], np.float32), consts)
    except Exception as ex:
        sys.stderr.write(f"[kernel] bass path failed ({ex!r}); numpy fallback\n")
        return _kernel_numpy(**inputs)


# revision 10
# speedup vs baseline: 2.1671x; 1.2298x over previous
import sys, os, hashlib
sys.path.insert(0, "/opt/trn_rl_repo")
import numpy as np
import ml_dtypes

DIM = 256; DIM_HEAD = 32; HEADS = 8; WSZ = 8; D4 = 64
EPS = 1e-5
SCALE = DIM_HEAD ** -0.5
NCORES = 8
HSH = 64  # H rows per core (one batch quarter)
BF16 = ml_dtypes.bfloat16
NEFF_CACHE_DIR = "/root/.bass_neff_cache"


def _ln_np(x, g, b):
    m = x.mean(-1, keepdims=True)
    v = x.var(-1, keepdims=True)
    return (x - m) / np.sqrt(v + EPS) * g + b


def _dpb_bias64(dpb_w1, dpb_b1, dpb_g1, dpb_beta1,
                dpb_w2, dpb_b2, dpb_g2, dpb_beta2,
                dpb_w3, dpb_b3, dpb_g3, dpb_beta3,
                dpb_w4, dpb_b4):
    pos = np.arange(-WSZ, WSZ + 1, dtype=np.float32)
    rel = np.stack(np.meshgrid(pos, pos, indexing='ij')).reshape(2, -1).T
    h = np.maximum(_ln_np(rel @ dpb_w1.T + dpb_b1, dpb_g1, dpb_beta1), 0)
    h = np.maximum(_ln_np(h @ dpb_w2.T + dpb_b2, dpb_g2, dpb_beta2), 0)
    h = np.maximum(_ln_np(h @ dpb_w3.T + dpb_b3, dpb_g3, dpb_beta3), 0)
    biases = (h @ dpb_w4.T + dpb_b4)[:, 0]
    p = np.arange(WSZ)
    grid = np.stack(np.meshgrid(p, p, indexing='ij')).reshape(2, -1).T
    r = grid[:, None] - grid[None, :] + WSZ - 1
    idx = r[..., 0] * (2 * WSZ - 1) + r[..., 1]
    return biases[idx].astype(np.float32)  # (64, 64)


def build_v2():
    from contextlib import ExitStack
    import concourse.bass as bass
    from concourse import mybir
    from concourse.tile import TileContext

    f32 = mybir.dt.float32
    bf16 = mybir.dt.bfloat16
    AX = mybir.AxisListType.X
    AF = mybir.ActivationFunctionType

    strips = [(hb, ws) for hb in range(8) for ws in range(4)]

    nc = bass.Bass(disable_frame_to_traceback=True)
    x_e = nc.declare_dram_parameter("x", [2, 128, HSH, 256], bf16, isOutput=False)
    wq_e = nc.declare_dram_parameter("wq", [2, 128, 768], bf16, isOutput=False)
    bq_e = nc.declare_dram_parameter("bq", [6, 128, 1], f32, isOutput=False)
    wo_e = nc.declare_dram_parameter("wo", [2, 128, 256], bf16, isOutput=False)
    bo_e = nc.declare_dram_parameter("bo", [2, 128, 1], f32, isOutput=False)
    bm_e = nc.declare_dram_parameter("biasm", [64, 512], f32, isOutput=False)
    idb_e = nc.declare_dram_parameter("idb", [128, 128], bf16, isOutput=False)
    out_e = nc.declare_dram_parameter("out", [2, 128, HSH, 256], bf16, isOutput=True)

    with TileContext(nc) as tc, ExitStack() as ctx:
        cpool = ctx.enter_context(tc.tile_pool(name="consts", bufs=1))
        wq = [cpool.tile([128, 768], bf16, tag=f"wq{i}", name=f"wq{i}") for i in range(2)]
        wo = [cpool.tile([128, 256], bf16, tag=f"wo{i}", name=f"wo{i}") for i in range(2)]
        bq6 = [cpool.tile([128, 1], f32, tag=f"bq{i}", name=f"bq{i}") for i in range(6)]
        bo2 = [cpool.tile([128, 1], f32, tag=f"bo{i}", name=f"bo{i}") for i in range(2)]
        biasm = cpool.tile([64, 512], f32, tag="biasm", name="biasm")
        idb = cpool.tile([128, 128], bf16, tag="idb", name="idb")
        onesb = cpool.tile([128, 128], bf16, tag="onesb", name="onesb")
        nc.vector.memset(onesb[:], 1.0)
        epsb = cpool.tile([128, 1], f32, tag="epsb", name="epsb")
        nc.vector.memset(epsb[:], EPS)
        for i in range(2):
            nc.sync.dma_start(out=wq[i][:], in_=wq_e[i])
            nc.sync.dma_start(out=wo[i][:], in_=wo_e[i])
            nc.sync.dma_start(out=bo2[i][:], in_=bo_e[i])
        for i in range(6):
            nc.sync.dma_start(out=bq6[i][:], in_=bq_e[i])
        nc.sync.dma_start(out=biasm[:], in_=bm_e[:])
        nc.sync.dma_start(out=idb[:], in_=idb_e[:])

        xpool = ctx.enter_context(tc.tile_pool(name="xp", bufs=2))
        spool = ctx.enter_context(tc.tile_pool(name="sp", bufs=2))
        qpool = ctx.enter_context(tc.tile_pool(name="qp", bufs=2))
        apool = ctx.enter_context(tc.tile_pool(name="ap", bufs=2))
        opool = ctx.enter_context(tc.tile_pool(name="op", bufs=2))
        p_st = ctx.enter_context(tc.tile_pool(name="pst", bufs=1, space="PSUM"))
        p_mm = ctx.enter_context(tc.tile_pool(name="pmm", bufs=2, space="PSUM"))
        p_sim = ctx.enter_context(tc.tile_pool(name="psim", bufs=2, space="PSUM"))
        p_tr = ctx.enter_context(tc.tile_pool(name="ptr", bufs=1, space="PSUM"))
        p_av = ctx.enter_context(tc.tile_pool(name="pav", bufs=1, space="PSUM"))

        for (hb, ws) in strips:
            h0, w0 = hb * 8, ws * 64
            # ---- load x rows (contiguous 256B runs per row)
            xt = [xpool.tile([128, 512], bf16, tag=f"xt{c}", name=f"xt{c}") for c in range(2)]
            sq = [xpool.tile([128, 512], bf16, tag=f"sq{c}", name=f"sq{c}") for c in range(2)]
            for c in range(2):
                src = x_e[c, :, h0:h0 + 8, w0:w0 + 64]
                nc.sync.dma_start(out=xt[c][:].rearrange("p (s1 w) -> p s1 w", s1=8), in_=src)
                nc.vector.tensor_mul(sq[c][:], xt[c][:], xt[c][:])
            # ---- channel stats via ones-matmul, replicated across partitions
            sm_ps = p_st.tile([128, 512], f32, tag="sm", name="sm")
            sq_ps = p_st.tile([128, 512], f32, tag="sqs", name="sqs")
            nc.tensor.matmul(sm_ps[:], onesb[:], xt[0][:], start=True, stop=False)
            nc.tensor.matmul(sm_ps[:], onesb[:], xt[1][:], start=False, stop=True)
            nc.tensor.matmul(sq_ps[:], onesb[:], sq[0][:], start=True, stop=False)
            nc.tensor.matmul(sq_ps[:], onesb[:], sq[1][:], start=False, stop=True)
            # ---- LN epilogue (walrus allows only one PSUM input per DVE op)
            ms = spool.tile([128, 512], f32, tag="ms", name="ms")
            nc.scalar.activation(ms[:], sm_ps[:], AF.Copy, scale=1.0 / 256.0)
            t = spool.tile([128, 512], f32, tag="t", name="t")
            nc.scalar.activation(t[:], sq_ps[:], AF.Copy, scale=1.0 / 256.0)
            msq = spool.tile([128, 512], f32, tag="msq", name="msq")
            nc.vector.tensor_mul(msq[:], ms[:], ms[:])
            d = spool.tile([128, 512], f32, tag="d", name="d")
            nc.vector.tensor_sub(d[:], t[:], msq[:])
            sr = spool.tile([128, 512], f32, tag="sr", name="sr")
            nc.scalar.activation(sr[:], d[:], AF.Sqrt, bias=epsb[:])
            rstd = spool.tile([128, 512], f32, tag="rstd", name="rstd")
            nc.vector.reciprocal(rstd[:], sr[:])
            # ---- z = (x - mean) * rstd
            z = [xpool.tile([128, 512], bf16, tag=f"z{c}", name=f"z{c}") for c in range(2)]
            for c in range(2):
                z0 = xpool.tile([128, 512], f32, tag=f"z0{c}", name=f"z0{c}")
                nc.vector.tensor_sub(z0[:], xt[c][:], ms[:])
                nc.vector.tensor_mul(z[c][:], z0[:], rstd[:])
            # ---- QKV projection; evacuate into window-major bf16 tiles.
            # HW erratum found empirically: a matmul operand slice at partition
            # base 32 of a COMPUTED tile wedges the device (DMA-sourced tiles
            # are fine). Store qkv per-head as [32, 512] tiles so every later
            # matmul operand sits at partition base 0.
            # tiles: q_h = h, k_h = 8+h, v_h = 16+h
            qkv = [qpool.tile([32, 512], bf16, tag=f"qkv{e}", name=f"qkv{e}") for e in range(24)]
            for e in range(6):
                ps = p_mm.tile([128, 512], f32, tag="mm", name="mm")
                nc.tensor.matmul(ps[:], wq[0][:, e * 128:(e + 1) * 128], z[0][:], start=True, stop=False)
                nc.tensor.matmul(ps[:], wq[1][:, e * 128:(e + 1) * 128], z[1][:], start=False, stop=True)
                sv = ps[:].rearrange("p (s1 ww s2) -> p s1 ww s2", s1=8, ww=8)
                for l in range(4):
                    dv = qkv[4 * e + l][:].rearrange("p (ww s1 s2) -> p s1 ww s2", ww=8, s1=8)
                    nc.vector.tensor_scalar_add(dv, sv[l * 32:(l + 1) * 32], bq6[e][l * 32:(l + 1) * 32])
            # ---- attention per window (all slices window-contiguous, base 0)
            ao = [apool.tile([128, 512], bf16, tag=f"ao{c}", name=f"ao{c}") for c in range(2)]
            for ww in range(8):
                wc = slice(ww * 64, ww * 64 + 64)
                sim_ps = p_sim.tile([64, 512], f32, tag="sim", name="sim")
                for h in range(HEADS):
                    nc.tensor.matmul(sim_ps[:, h * 64:(h + 1) * 64],
                                     qkv[h][:, wc],
                                     qkv[8 + h][:, wc],
                                     start=True, stop=True)
                at = apool.tile([64, 512], f32, tag="at", name="at")
                nc.vector.tensor_add(at[:], sim_ps[:], biasm[:])
                A = apool.tile([64, 512], bf16, tag="A", name="A")
                nc.scalar.activation(A[:], at[:], AF.Exp)
                dn = spool.tile([64, 8], f32, tag="dn", name="dn")
                nc.vector.reduce_sum(dn[:], A[:].rearrange("p (h j) -> p h j", h=8), axis=AX)
                rc = spool.tile([64, 8], f32, tag="rc", name="rc")
                nc.vector.reciprocal(rc[:], dn[:])
                An = apool.tile([64, 512], bf16, tag="An", name="An")
                nc.vector.tensor_mul(
                    An[:].rearrange("p (h j) -> p h j", h=8),
                    A[:].rearrange("p (h j) -> p h j", h=8),
                    rc[:].unsqueeze(2).broadcast_to([64, 8, 64]))
                # A^T per head-pair transpose [64, 128] -> [128, 64], then split
                # into per-head [64, 64] tiles at partition base 0
                aT = [apool.tile([64, 64], bf16, tag=f"aT{h}", name=f"aT{h}") for h in range(8)]
                for p in range(4):
                    tp = p_tr.tile([128, 64], bf16, tag="trp", name="trp")
                    nc.tensor.transpose(tp[:], An[:, p * 128:(p + 1) * 128], idb[0:64, 0:64])
                    nc.scalar.copy(aT[2 * p][:], tp[0:64, :])
                    nc.scalar.copy(aT[2 * p + 1][:], tp[64:128, :])
                # V^T per head: [32, 64] -> [64, 32], packed [64, 256]
                vT = apool.tile([64, 256], bf16, tag="vT", name="vT")
                for h in range(HEADS):
                    tv = p_tr.tile([64, 64], bf16, tag="trp", name="trp")
                    nc.tensor.transpose(tv[:, 0:32], qkv[16 + h][:, wc], idb[0:32, 0:32])
                    nc.scalar.copy(vT[:, h * 32:(h + 1) * 32], tv[:, 0:32])
                # out2 = V^T A^T, heads packed along columns (no PSUM group overlap)
                av_ps = p_av.tile([32, 512], f32, tag="av", name="av")
                for h in range(HEADS):
                    nc.tensor.matmul(av_ps[:, h * 64:(h + 1) * 64],
                                     vT[:, h * 32:(h + 1) * 32],
                                     aT[h][:],
                                     start=True, stop=True)
                for h in range(HEADS):
                    c, r = h // 4, (h % 4) * 32
                    nc.scalar.copy(ao[c][r:r + 32, wc], av_ps[:, h * 64:(h + 1) * 64])
            # ---- output projection; un-window on evacuation; store
            for c in range(2):
                ps = p_mm.tile([128, 512], f32, tag="mm", name="mm")
                nc.tensor.matmul(ps[:], wo[0][:, c * 128:(c + 1) * 128], ao[0][:], start=True, stop=False)
                nc.tensor.matmul(ps[:], wo[1][:, c * 128:(c + 1) * 128], ao[1][:], start=False, stop=True)
                orm = opool.tile([128, 512], bf16, tag=f"orm{c}", name=f"orm{c}")
                dv = orm[:].rearrange("p (s1 ww s2) -> p ww s1 s2", s1=8, ww=8)
                sv = ps[:].rearrange("p (ww s1 s2) -> p ww s1 s2", ww=8, s1=8)
                nc.vector.tensor_scalar_add(dv, sv, bo2[c][:])
                nc.sync.dma_start(out=out_e[c, :, h0:h0 + 8, w0:w0 + 64],
                                  in_=orm[:].rearrange("p (s1 w) -> p s1 w", s1=8))
    return nc


def _split_multi_waits(nc, max_waits=1):
    # walrus codegen in this container rejects instructions carrying more
    # than one sem-wait ("Too many sync wait commands"). Move excess waits
    # onto InstNoOp carriers inserted just before, on the same engine
    # (engine queues are in-order, so semantics are preserved).
    from concourse import mybir
    n_split = 0
    for fn in nc.m.functions:
        for blk in fn.blocks:
            insts = blk.instructions
            i = 0
            while i < len(insts):
                inst = insts[i]
                si = inst.sync_info
                if si is not None and si.on_wait and len(si.on_wait) > max_waits:
                    waits = list(si.on_wait)
                    keep = waits[-max_waits:]
                    extra = waits[:-max_waits]
                    carriers = []
                    for j in range(0, len(extra), max_waits):
                        chunk = extra[j:j + max_waits]
                        nop = mybir.InstNoOp(
                            name=nc.get_next_instruction_name(),
                            sync_info=mybir.SyncInfo(on_wait=chunk, on_update=[]),
                            bass_nofuse=True,
                            engine=inst.engine,
                        )
                        nc.register_instruction(nop)
                        carriers.append(nop)
                    inst.sync_info = mybir.SyncInfo(
                        on_wait=keep, on_update=list(si.on_update or [])
                    )
                    insts[i:i] = carriers
                    i += len(carriers)
                    n_split += 1
                i += 1
    return n_split


def _install_neff_disk_cache():
    # cache walrus-compiled NEFFs on disk keyed by BIR bytes, so repeat runs
    # (including fresh processes) skip the multi-minute backend compile
    import concourse.bass2jax as b2j
    if getattr(b2j, "_neff_cache_installed", False):
        return
    orig = b2j.compile_bir_kernel

    def cached(bir_json, tmpdir, neff_name="file.neff"):
        try:
            os.makedirs(NEFF_CACHE_DIR, exist_ok=True)
            key = hashlib.sha256(bir_json).hexdigest()
            path = os.path.join(NEFF_CACHE_DIR, key + ".neff")
            if os.path.exists(path):
                dst = os.path.join(tmpdir, neff_name)
                with open(path, "rb") as f, open(dst, "wb") as g:
                    g.write(f.read())
                return dst
            out = orig(bir_json, tmpdir, neff_name)
            with open(out, "rb") as f:
                data = f.read()
            tmp = path + ".tmp"
            with open(tmp, "wb") as f:
                f.write(data)
            os.replace(tmp, path)
            return out
        except Exception:
            return orig(bir_json, tmpdir, neff_name)

    b2j.compile_bir_kernel = cached
    b2j._neff_cache_installed = True


def prep_consts(norm_g, norm_b, w_qkv, w_out, b_out, **dpb):
    g = np.asarray(norm_g, np.float32).reshape(DIM)
    bvec = np.asarray(norm_b, np.float32).reshape(DIM)
    W = np.asarray(w_qkv, np.float32)
    Wg = W * g[None, :]
    Wg[:256] *= SCALE
    bq = W @ bvec
    bq = bq.copy(); bq[:256] *= SCALE
    wq = np.ascontiguousarray(Wg.T.reshape(2, 128, 768)).astype(BF16)
    bq6 = np.ascontiguousarray(bq.reshape(6, 128, 1)).astype(np.float32)
    wo = np.ascontiguousarray(np.asarray(w_out, np.float32).T.reshape(2, 128, 256)).astype(BF16)
    bo = np.ascontiguousarray(np.asarray(b_out, np.float32).reshape(2, 128, 1))
    bias64 = _dpb_bias64(**{k: np.asarray(v, np.float32) for k, v in dpb.items()})
    biasm = np.ascontiguousarray(np.tile(bias64, (1, 8)))
    idb = np.eye(128).astype(BF16)
    return dict(wq=wq, bq=bq6, wo=wo, bo=bo, biasm=biasm, idb=idb)


LAST = None

# order matches build_v2's declare_dram_parameter calls (x first, out excluded)
IN_NAMES = ["x", "wq", "bq", "wo", "bo", "biasm", "idb"]
EXPORT_VERSION = "v3"


def _export_cache_path():
    import inspect
    key = hashlib.sha256(
        (inspect.getsource(build_v2) + EXPORT_VERSION).encode()).hexdigest()[:24]
    return os.path.join(NEFF_CACHE_DIR, f"export_{key}.bin")


def _patch_bass_effect():
    import concourse.bass2jax as b2j
    # jax.export requires effects to be reconstructible via a nullary
    # constructor producing an equal object; BassEffect is stateless
    b2j.BassEffect.__eq__ = lambda self, other: isinstance(other, b2j.BassEffect)
    b2j.BassEffect.__hash__ = lambda self: hash(b2j.BassEffect)


def _make_exported():
    # build the bass module and export the lowered sharded call (BIR is
    # embedded in the custom-call backend_config, so the deserialized module
    # no longer needs bass at all; output zeros are created on-device inside)
    import jax
    import jax.export
    import jax.numpy as jnp
    from jax.experimental.shard_map import shard_map
    from jax.sharding import Mesh, PartitionSpec
    from concourse import mybir
    import concourse.bass2jax as b2j

    nc = build_v2()
    _split_multi_waits(nc)
    b2j.install_neuronx_cc_hook()
    _patch_bass_effect()

    partition_name = nc.partition_id_tensor.name if nc.partition_id_tensor else None
    in_names, out_names, out_avals = [], [], []
    for alloc in nc.m.functions[0].allocations:
        if not isinstance(alloc, mybir.MemoryLocationSet):
            continue
        name = alloc.memorylocations[0].name
        if alloc.kind == "ExternalInput":
            if name != partition_name:
                in_names.append(name)
        elif alloc.kind == "ExternalOutput":
            out_names.append(name)
            out_avals.append(jax.core.ShapedArray(tuple(alloc.tensor_shape),
                                                  mybir.dt.np(alloc.dtype)))
    assert in_names == IN_NAMES, in_names
    all_names = list(in_names) + list(out_names)
    if partition_name is not None:
        all_names.append(partition_name)

    def _body(*args):
        operands = list(args)
        if partition_name is not None:
            operands.append(b2j.partition_id_tensor())
        outs = b2j._bass_exec_p.bind(
            *operands,
            out_avals=tuple(out_avals),
            in_names=tuple(all_names),
            out_names=tuple(out_names),
            lowering_input_output_aliases=(),
            sim_require_finite=True,
            sim_require_nnan=True,
            nc=nc,
        )
        return tuple(outs)

    mesh = Mesh(np.asarray(jax.devices()[:NCORES]), ("core",))
    n_io = len(in_names) + len(out_names)
    sharded = jax.jit(
        shard_map(_body, mesh=mesh,
                  in_specs=(PartitionSpec("core"),) * n_io,
                  out_specs=(PartitionSpec("core"),) * len(out_names),
                  check_rep=False))
    shapes = {"x": (2, 128, HSH, 256), "wq": (2, 128, 768), "bq": (6, 128, 1),
              "wo": (2, 128, 256), "bo": (2, 128, 1), "biasm": (64, 512),
              "idb": (128, 128)}
    dts = {"x": BF16, "wq": BF16, "bq": np.float32, "wo": BF16,
           "bo": np.float32, "biasm": np.float32, "idb": BF16}
    args = [jax.ShapeDtypeStruct((NCORES * shapes[nm][0], *shapes[nm][1:]), dts[nm])
            for nm in in_names]
    zargs = [jax.ShapeDtypeStruct((NCORES * a.shape[0], *a.shape[1:]), a.dtype)
             for a in out_avals]
    dsc = jax.export.DisabledSafetyCheck.custom_call("bass_exec")
    return jax.export.export(sharded, disabled_checks=[dsc])(*args, *zargs)


def _get_exported():
    import jax.export
    _install_neff_disk_cache()
    _patch_bass_effect()
    path = _export_cache_path()
    if os.path.exists(path):
        try:
            return jax.export.deserialize(open(path, "rb").read())
        except Exception:
            pass
    exp = _make_exported()
    try:
        os.makedirs(NEFF_CACHE_DIR, exist_ok=True)
        tmp = path + ".tmp"
        with open(tmp, "wb") as f:
            f.write(exp.serialize())
        os.replace(tmp, path)
    except Exception:
        pass
    return exp


def _kernel_bass(x, consts):
    global LAST
    import jax
    exp = _get_exported()
    import concourse.bass2jax as b2j
    b2j.install_neuronx_cc_hook()

    xs = np.concatenate([
        np.ascontiguousarray(x[i // 4, :, (i % 4) * 64:(i % 4) * 64 + 64, :])
        .reshape(2, 128, HSH, 256).astype(BF16)
        for i in range(NCORES)], axis=0)
    gin = [xs] + [np.concatenate([consts[nm]] * NCORES, axis=0)
                  for nm in IN_NAMES[1:]]
    import jax.numpy as jnp
    from jax.sharding import Mesh, PartitionSpec, NamedSharding
    mesh = Mesh(np.asarray(jax.devices()[:NCORES]), ("core",))
    sh = NamedSharding(mesh, PartitionSpec("core"))
    zfn = jax.jit(lambda: jnp.zeros((NCORES * 2, 128, HSH, 256), BF16),
                  out_shardings=sh)
    z = zfn()
    out = jax.jit(exp.call, in_shardings=(sh,) * (len(gin) + 1),
                  out_shardings=sh)(*gin, z)
    out0 = np.asarray(out[0] if isinstance(out, (tuple, list)) else out)
    res = out0.reshape(NCORES, 2, 128, HSH, 256)
    full = np.empty((2, DIM, 256, 256), dtype=np.float32)
    for i in range(NCORES):
        b, r0 = i // 4, (i % 4) * 64
        full[b, :, r0:r0 + 64, :] = res[i].reshape(256, 64, 256).astype(np.float32)
    return full


def _kernel_numpy(x, norm_g, norm_b, w_qkv, w_out, b_out, **dpb):
    # fallback: straight port of the reference in numpy (f32)
    B, D, H, W = x.shape
    nh, nw = H // WSZ, W // WSZ
    mean = x.mean(axis=1, keepdims=True)
    var = x.var(axis=1, keepdims=True)
    xn = (x - mean) / np.sqrt(var + EPS) * norm_g + norm_b
    xw = xn.reshape(B, D, nh, WSZ, nw, WSZ).transpose(0, 2, 4, 1, 3, 5)
    xw = xw.reshape(B * nh * nw, D, WSZ * WSZ)
    qkv = np.einsum('ed,bdn->ben', w_qkv, xw)
    q, k, v = np.split(qkv, 3, axis=1)
    th = lambda t: t.reshape(-1, HEADS, DIM_HEAD, WSZ * WSZ).transpose(0, 1, 3, 2)
    q, k, v = th(q) * SCALE, th(k), th(v)
    sim = np.einsum('bhid,bhjd->bhij', q, k)
    sim = sim + _dpb_bias64(**dpb)[None, None]
    sim = sim - sim.max(-1, keepdims=True)
    e = np.exp(sim)
    attn = e / e.sum(-1, keepdims=True)
    o = np.einsum('bhij,bhjd->bhid', attn, v)
    o = o.transpose(0, 1, 3, 2).reshape(-1, HEADS * DIM_HEAD, WSZ * WSZ)
    o = np.einsum('de,ben->bdn', w_out, o) + b_out[None, :, None]
    o = o.reshape(B, nh, nw, D, WSZ, WSZ).transpose(0, 3, 1, 4, 2, 5).reshape(B, D, H, W)
    return o.astype(np.float32)


def kernel(**inputs):
    inputs = {k: np.asarray(v) for k, v in inputs.items()}
    try:
        consts = prep_consts(**{k: v for k, v in inputs.items() if k != "x"})
        return _kernel_bass(np.asarray(inputs["x"], np.float32), consts)
    except Exception as ex:
        sys.stderr.write(f"[kernel] bass path failed ({ex!r}); numpy fallback\n")
        return _kernel_numpy(**inputs)


# revision 11
# speedup vs baseline: 11.4377x; 5.2780x over previous
import sys, os, hashlib
sys.path.insert(0, "/opt/trn_rl_repo")
import numpy as np
import ml_dtypes

DIM = 256; DIM_HEAD = 32; HEADS = 8; WSZ = 8; D4 = 64
EPS = 1e-5
SCALE = DIM_HEAD ** -0.5
NCORES = 8
HSH = 64  # H rows per core (one batch quarter)
BF16 = ml_dtypes.bfloat16
NEFF_CACHE_DIR = "/root/.bass_neff_cache"


def _ln_np(x, g, b):
    m = x.mean(-1, keepdims=True)
    v = x.var(-1, keepdims=True)
    return (x - m) / np.sqrt(v + EPS) * g + b


def _dpb_bias64(dpb_w1, dpb_b1, dpb_g1, dpb_beta1,
                dpb_w2, dpb_b2, dpb_g2, dpb_beta2,
                dpb_w3, dpb_b3, dpb_g3, dpb_beta3,
                dpb_w4, dpb_b4):
    pos = np.arange(-WSZ, WSZ + 1, dtype=np.float32)
    rel = np.stack(np.meshgrid(pos, pos, indexing='ij')).reshape(2, -1).T
    h = np.maximum(_ln_np(rel @ dpb_w1.T + dpb_b1, dpb_g1, dpb_beta1), 0)
    h = np.maximum(_ln_np(h @ dpb_w2.T + dpb_b2, dpb_g2, dpb_beta2), 0)
    h = np.maximum(_ln_np(h @ dpb_w3.T + dpb_b3, dpb_g3, dpb_beta3), 0)
    biases = (h @ dpb_w4.T + dpb_b4)[:, 0]
    p = np.arange(WSZ)
    grid = np.stack(np.meshgrid(p, p, indexing='ij')).reshape(2, -1).T
    r = grid[:, None] - grid[None, :] + WSZ - 1
    idx = r[..., 0] * (2 * WSZ - 1) + r[..., 1]
    return biases[idx].astype(np.float32)  # (64, 64)


def build_v2():
    from contextlib import ExitStack
    import concourse.bass as bass
    from concourse import mybir
    from concourse.tile import TileContext

    f32 = mybir.dt.float32
    bf16 = mybir.dt.bfloat16
    AX = mybir.AxisListType.X
    AF = mybir.ActivationFunctionType

    strips = [(hb, ws) for hb in range(8) for ws in range(4)]

    nc = bass.Bass(disable_frame_to_traceback=True)
    x_e = nc.declare_dram_parameter("x", [2, 128, HSH, 256], bf16, isOutput=False)
    wq_e = nc.declare_dram_parameter("wq", [2, 128, 768], bf16, isOutput=False)
    bq_e = nc.declare_dram_parameter("bq", [6, 128, 1], f32, isOutput=False)
    wo_e = nc.declare_dram_parameter("wo", [2, 128, 256], bf16, isOutput=False)
    bo_e = nc.declare_dram_parameter("bo", [2, 128, 1], f32, isOutput=False)
    bm_e = nc.declare_dram_parameter("biasm", [64, 512], f32, isOutput=False)
    idb_e = nc.declare_dram_parameter("idb", [128, 128], bf16, isOutput=False)
    out_e = nc.declare_dram_parameter("out", [2, 128, HSH, 256], bf16, isOutput=True)

    with TileContext(nc) as tc, ExitStack() as ctx:
        cpool = ctx.enter_context(tc.tile_pool(name="consts", bufs=1))
        wq = [cpool.tile([128, 768], bf16, tag=f"wq{i}", name=f"wq{i}") for i in range(2)]
        wo = [cpool.tile([128, 256], bf16, tag=f"wo{i}", name=f"wo{i}") for i in range(2)]
        bq6 = [cpool.tile([128, 1], f32, tag=f"bq{i}", name=f"bq{i}") for i in range(6)]
        bo2 = [cpool.tile([128, 1], f32, tag=f"bo{i}", name=f"bo{i}") for i in range(2)]
        biasm = cpool.tile([64, 512], f32, tag="biasm", name="biasm")
        idb = cpool.tile([128, 128], bf16, tag="idb", name="idb")
        onesb = cpool.tile([128, 128], bf16, tag="onesb", name="onesb")
        nc.vector.memset(onesb[:], 1.0)
        epsb = cpool.tile([128, 1], f32, tag="epsb", name="epsb")
        nc.vector.memset(epsb[:], EPS)
        for i in range(2):
            nc.sync.dma_start(out=wq[i][:], in_=wq_e[i])
            nc.sync.dma_start(out=wo[i][:], in_=wo_e[i])
            nc.sync.dma_start(out=bo2[i][:], in_=bo_e[i])
        for i in range(6):
            nc.sync.dma_start(out=bq6[i][:], in_=bq_e[i])
        nc.sync.dma_start(out=biasm[:], in_=bm_e[:])
        nc.sync.dma_start(out=idb[:], in_=idb_e[:])

        xpool = ctx.enter_context(tc.tile_pool(name="xp", bufs=2))
        spool = ctx.enter_context(tc.tile_pool(name="sp", bufs=2))
        qpool = ctx.enter_context(tc.tile_pool(name="qp", bufs=2))
        apool = ctx.enter_context(tc.tile_pool(name="ap", bufs=2))
        opool = ctx.enter_context(tc.tile_pool(name="op", bufs=2))
        p_st = ctx.enter_context(tc.tile_pool(name="pst", bufs=1, space="PSUM"))
        p_mm = ctx.enter_context(tc.tile_pool(name="pmm", bufs=2, space="PSUM"))
        p_sim = ctx.enter_context(tc.tile_pool(name="psim", bufs=2, space="PSUM"))
        p_tr = ctx.enter_context(tc.tile_pool(name="ptr", bufs=1, space="PSUM"))
        p_av = ctx.enter_context(tc.tile_pool(name="pav", bufs=1, space="PSUM"))

        for (hb, ws) in strips:
            h0, w0 = hb * 8, ws * 64
            # ---- load x rows (contiguous 256B runs per row)
            xt = [xpool.tile([128, 512], bf16, tag=f"xt{c}", name=f"xt{c}") for c in range(2)]
            sq = [xpool.tile([128, 512], bf16, tag=f"sq{c}", name=f"sq{c}") for c in range(2)]
            for c in range(2):
                src = x_e[c, :, h0:h0 + 8, w0:w0 + 64]
                nc.sync.dma_start(out=xt[c][:].rearrange("p (s1 w) -> p s1 w", s1=8), in_=src)
                nc.vector.tensor_mul(sq[c][:], xt[c][:], xt[c][:])
            # ---- channel stats via ones-matmul, replicated across partitions
            sm_ps = p_st.tile([128, 512], f32, tag="sm", name="sm")
            sq_ps = p_st.tile([128, 512], f32, tag="sqs", name="sqs")
            nc.tensor.matmul(sm_ps[:], onesb[:], xt[0][:], start=True, stop=False)
            nc.tensor.matmul(sm_ps[:], onesb[:], xt[1][:], start=False, stop=True)
            nc.tensor.matmul(sq_ps[:], onesb[:], sq[0][:], start=True, stop=False)
            nc.tensor.matmul(sq_ps[:], onesb[:], sq[1][:], start=False, stop=True)
            # ---- LN epilogue (walrus allows only one PSUM input per DVE op)
            ms = spool.tile([128, 512], f32, tag="ms", name="ms")
            nc.scalar.activation(ms[:], sm_ps[:], AF.Copy, scale=1.0 / 256.0)
            t = spool.tile([128, 512], f32, tag="t", name="t")
            nc.scalar.activation(t[:], sq_ps[:], AF.Copy, scale=1.0 / 256.0)
            msq = spool.tile([128, 512], f32, tag="msq", name="msq")
            nc.vector.tensor_mul(msq[:], ms[:], ms[:])
            d = spool.tile([128, 512], f32, tag="d", name="d")
            nc.vector.tensor_sub(d[:], t[:], msq[:])
            sr = spool.tile([128, 512], f32, tag="sr", name="sr")
            nc.scalar.activation(sr[:], d[:], AF.Sqrt, bias=epsb[:])
            rstd = spool.tile([128, 512], f32, tag="rstd", name="rstd")
            nc.vector.reciprocal(rstd[:], sr[:])
            # ---- z = (x - mean) * rstd
            z = [xpool.tile([128, 512], bf16, tag=f"z{c}", name=f"z{c}") for c in range(2)]
            for c in range(2):
                z0 = xpool.tile([128, 512], f32, tag=f"z0{c}", name=f"z0{c}")
                nc.vector.tensor_sub(z0[:], xt[c][:], ms[:])
                nc.vector.tensor_mul(z[c][:], z0[:], rstd[:])
            # ---- QKV projection; evacuate into window-major bf16 tiles.
            # HW erratum found empirically: a matmul operand slice at partition
            # base 32 of a COMPUTED tile wedges the device (DMA-sourced tiles
            # are fine). Store qkv per-head as [32, 512] tiles so every later
            # matmul operand sits at partition base 0.
            # tiles: q_h = h, k_h = 8+h, v_h = 16+h
            qkv = [qpool.tile([32, 512], bf16, tag=f"qkv{e}", name=f"qkv{e}") for e in range(24)]
            for e in range(6):
                ps = p_mm.tile([128, 512], f32, tag="mm", name="mm")
                nc.tensor.matmul(ps[:], wq[0][:, e * 128:(e + 1) * 128], z[0][:], start=True, stop=False)
                nc.tensor.matmul(ps[:], wq[1][:, e * 128:(e + 1) * 128], z[1][:], start=False, stop=True)
                sv = ps[:].rearrange("p (s1 ww s2) -> p s1 ww s2", s1=8, ww=8)
                for l in range(4):
                    dv = qkv[4 * e + l][:].rearrange("p (ww s1 s2) -> p s1 ww s2", ww=8, s1=8)
                    nc.vector.tensor_scalar_add(dv, sv[l * 32:(l + 1) * 32], bq6[e][l * 32:(l + 1) * 32])
            # ---- attention per window (all slices window-contiguous, base 0)
            ao = [apool.tile([128, 512], bf16, tag=f"ao{c}", name=f"ao{c}") for c in range(2)]
            for ww in range(8):
                wc = slice(ww * 64, ww * 64 + 64)
                sim_ps = p_sim.tile([64, 512], f32, tag="sim", name="sim")
                for h in range(HEADS):
                    nc.tensor.matmul(sim_ps[:, h * 64:(h + 1) * 64],
                                     qkv[h][:, wc],
                                     qkv[8 + h][:, wc],
                                     start=True, stop=True)
                at = apool.tile([64, 512], f32, tag="at", name="at")
                nc.vector.tensor_add(at[:], sim_ps[:], biasm[:])
                A = apool.tile([64, 512], bf16, tag="A", name="A")
                nc.scalar.activation(A[:], at[:], AF.Exp)
                dn = spool.tile([64, 8], f32, tag="dn", name="dn")
                nc.vector.reduce_sum(dn[:], A[:].rearrange("p (h j) -> p h j", h=8), axis=AX)
                rc = spool.tile([64, 8], f32, tag="rc", name="rc")
                nc.vector.reciprocal(rc[:], dn[:])
                An = apool.tile([64, 512], bf16, tag="An", name="An")
                nc.vector.tensor_mul(
                    An[:].rearrange("p (h j) -> p h j", h=8),
                    A[:].rearrange("p (h j) -> p h j", h=8),
                    rc[:].unsqueeze(2).broadcast_to([64, 8, 64]))
                # A^T per head-pair transpose [64, 128] -> [128, 64], then split
                # into per-head [64, 64] tiles at partition base 0
                aT = [apool.tile([64, 64], bf16, tag=f"aT{h}", name=f"aT{h}") for h in range(8)]
                for p in range(4):
                    tp = p_tr.tile([128, 64], bf16, tag="trp", name="trp")
                    nc.tensor.transpose(tp[:], An[:, p * 128:(p + 1) * 128], idb[0:64, 0:64])
                    nc.scalar.copy(aT[2 * p][:], tp[0:64, :])
                    nc.scalar.copy(aT[2 * p + 1][:], tp[64:128, :])
                # V^T per head: [32, 64] -> [64, 32], packed [64, 256]
                vT = apool.tile([64, 256], bf16, tag="vT", name="vT")
                for h in range(HEADS):
                    tv = p_tr.tile([64, 64], bf16, tag="trp", name="trp")
                    nc.tensor.transpose(tv[:, 0:32], qkv[16 + h][:, wc], idb[0:32, 0:32])
                    nc.scalar.copy(vT[:, h * 32:(h + 1) * 32], tv[:, 0:32])
                # out2 = V^T A^T, heads packed along columns (no PSUM group overlap)
                av_ps = p_av.tile([32, 512], f32, tag="av", name="av")
                for h in range(HEADS):
                    nc.tensor.matmul(av_ps[:, h * 64:(h + 1) * 64],
                                     vT[:, h * 32:(h + 1) * 32],
                                     aT[h][:],
                                     start=True, stop=True)
                for h in range(HEADS):
                    c, r = h // 4, (h % 4) * 32
                    nc.scalar.copy(ao[c][r:r + 32, wc], av_ps[:, h * 64:(h + 1) * 64])
            # ---- output projection; un-window on evacuation; store
            for c in range(2):
                ps = p_mm.tile([128, 512], f32, tag="mm", name="mm")
                nc.tensor.matmul(ps[:], wo[0][:, c * 128:(c + 1) * 128], ao[0][:], start=True, stop=False)
                nc.tensor.matmul(ps[:], wo[1][:, c * 128:(c + 1) * 128], ao[1][:], start=False, stop=True)
                orm = opool.tile([128, 512], bf16, tag=f"orm{c}", name=f"orm{c}")
                dv = orm[:].rearrange("p (s1 ww s2) -> p ww s1 s2", s1=8, ww=8)
                sv = ps[:].rearrange("p (ww s1 s2) -> p ww s1 s2", ww=8, s1=8)
                nc.vector.tensor_scalar_add(dv, sv, bo2[c][:])
                nc.sync.dma_start(out=out_e[c, :, h0:h0 + 8, w0:w0 + 64],
                                  in_=orm[:].rearrange("p (s1 w) -> p s1 w", s1=8))
    return nc


def _split_multi_waits(nc, max_waits=1):
    # walrus codegen in this container rejects instructions carrying more
    # than one sem-wait ("Too many sync wait commands"). Move excess waits
    # onto InstNoOp carriers inserted just before, on the same engine
    # (engine queues are in-order, so semantics are preserved).
    from concourse import mybir
    n_split = 0
    for fn in nc.m.functions:
        for blk in fn.blocks:
            insts = blk.instructions
            i = 0
            while i < len(insts):
                inst = insts[i]
                si = inst.sync_info
                if si is not None and si.on_wait and len(si.on_wait) > max_waits:
                    waits = list(si.on_wait)
                    keep = waits[-max_waits:]
                    extra = waits[:-max_waits]
                    carriers = []
                    for j in range(0, len(extra), max_waits):
                        chunk = extra[j:j + max_waits]
                        nop = mybir.InstNoOp(
                            name=nc.get_next_instruction_name(),
                            sync_info=mybir.SyncInfo(on_wait=chunk, on_update=[]),
                            bass_nofuse=True,
                            engine=inst.engine,
                        )
                        nc.register_instruction(nop)
                        carriers.append(nop)
                    inst.sync_info = mybir.SyncInfo(
                        on_wait=keep, on_update=list(si.on_update or [])
                    )
                    insts[i:i] = carriers
                    i += len(carriers)
                    n_split += 1
                i += 1
    return n_split


def _install_neff_disk_cache():
    # cache walrus-compiled NEFFs on disk keyed by BIR bytes, so repeat runs
    # (including fresh processes) skip the multi-minute backend compile
    import concourse.bass2jax as b2j
    if getattr(b2j, "_neff_cache_installed", False):
        return
    orig = b2j.compile_bir_kernel

    def cached(bir_json, tmpdir, neff_name="file.neff"):
        try:
            os.makedirs(NEFF_CACHE_DIR, exist_ok=True)
            key = hashlib.sha256(bir_json).hexdigest()
            path = os.path.join(NEFF_CACHE_DIR, key + ".neff")
            if os.path.exists(path):
                dst = os.path.join(tmpdir, neff_name)
                with open(path, "rb") as f, open(dst, "wb") as g:
                    g.write(f.read())
                return dst
            out = orig(bir_json, tmpdir, neff_name)
            with open(out, "rb") as f:
                data = f.read()
            tmp = path + ".tmp"
            with open(tmp, "wb") as f:
                f.write(data)
            os.replace(tmp, path)
            return out
        except Exception:
            return orig(bir_json, tmpdir, neff_name)

    b2j.compile_bir_kernel = cached
    b2j._neff_cache_installed = True


def prep_consts(norm_g, norm_b, w_qkv, w_out, b_out, **dpb):
    g = np.asarray(norm_g, np.float32).reshape(DIM)
    bvec = np.asarray(norm_b, np.float32).reshape(DIM)
    W = np.asarray(w_qkv, np.float32)
    Wg = W * g[None, :]
    Wg[:256] *= SCALE
    bq = W @ bvec
    bq = bq.copy(); bq[:256] *= SCALE
    wq = np.ascontiguousarray(Wg.T.reshape(2, 128, 768)).astype(BF16)
    bq6 = np.ascontiguousarray(bq.reshape(6, 128, 1)).astype(np.float32)
    wo = np.ascontiguousarray(np.asarray(w_out, np.float32).T.reshape(2, 128, 256)).astype(BF16)
    bo = np.ascontiguousarray(np.asarray(b_out, np.float32).reshape(2, 128, 1))
    bias64 = _dpb_bias64(**{k: np.asarray(v, np.float32) for k, v in dpb.items()})
    biasm = np.ascontiguousarray(np.tile(bias64, (1, 8)))
    idb = np.eye(128).astype(BF16)
    return dict(wq=wq, bq=bq6, wo=wo, bo=bo, biasm=biasm, idb=idb)


LAST = None

# order matches build_v2's declare_dram_parameter calls (x first, out excluded)
IN_NAMES = ["x", "wq", "bq", "wo", "bo", "biasm", "idb"]
EXPORT_VERSION = "v3"


def _export_cache_path():
    import inspect
    key = hashlib.sha256(
        (inspect.getsource(build_v2) + inspect.getsource(_make_exported)
         + EXPORT_VERSION).encode()).hexdigest()[:24]
    return os.path.join(NEFF_CACHE_DIR, f"export_{key}.bin")


def _patch_bass_effect():
    import concourse.bass2jax as b2j
    # jax.export requires effects to be reconstructible via a nullary
    # constructor producing an equal object; BassEffect is stateless
    b2j.BassEffect.__eq__ = lambda self, other: isinstance(other, b2j.BassEffect)
    b2j.BassEffect.__hash__ = lambda self: hash(b2j.BassEffect)


def _make_exported():
    # build the bass module and export the lowered sharded call (BIR is
    # embedded in the custom-call backend_config, so the deserialized module
    # no longer needs bass at all; output zeros are created on-device inside)
    import jax
    import jax.export
    import jax.numpy as jnp
    from jax.experimental.shard_map import shard_map
    from jax.sharding import Mesh, PartitionSpec
    from concourse import mybir
    import concourse.bass2jax as b2j

    nc = build_v2()
    _split_multi_waits(nc)
    b2j.install_neuronx_cc_hook()
    _patch_bass_effect()

    partition_name = nc.partition_id_tensor.name if nc.partition_id_tensor else None
    in_names, out_names, out_avals = [], [], []
    for alloc in nc.m.functions[0].allocations:
        if not isinstance(alloc, mybir.MemoryLocationSet):
            continue
        name = alloc.memorylocations[0].name
        if alloc.kind == "ExternalInput":
            if name != partition_name:
                in_names.append(name)
        elif alloc.kind == "ExternalOutput":
            out_names.append(name)
            out_avals.append(jax.core.ShapedArray(tuple(alloc.tensor_shape),
                                                  mybir.dt.np(alloc.dtype)))
    assert in_names == IN_NAMES, in_names
    all_names = list(in_names) + list(out_names)
    if partition_name is not None:
        all_names.append(partition_name)

    def _body(*args):
        operands = list(args)
        if partition_name is not None:
            operands.append(b2j.partition_id_tensor())
        outs = b2j._bass_exec_p.bind(
            *operands,
            out_avals=tuple(out_avals),
            in_names=tuple(all_names),
            out_names=tuple(out_names),
            lowering_input_output_aliases=(),
            sim_require_finite=True,
            sim_require_nnan=True,
            nc=nc,
        )
        return tuple(outs)

    mesh = Mesh(np.asarray(jax.devices()[:NCORES]), ("core",))
    n_io = len(in_names) + len(out_names)
    sharded = jax.jit(
        shard_map(_body, mesh=mesh,
                  in_specs=(PartitionSpec("core"),) * n_io,
                  out_specs=(PartitionSpec("core"),) * len(out_names),
                  check_rep=False))
    shapes = {"x": (2, 128, HSH, 256), "wq": (2, 128, 768), "bq": (6, 128, 1),
              "wo": (2, 128, 256), "bo": (2, 128, 1), "biasm": (64, 512),
              "idb": (128, 128)}
    dts = {"x": BF16, "wq": BF16, "bq": np.float32, "wo": BF16,
           "bo": np.float32, "biasm": np.float32, "idb": BF16}
    args = [jax.ShapeDtypeStruct((NCORES * shapes[nm][0], *shapes[nm][1:]), dts[nm])
            for nm in in_names]
    zargs = [jax.ShapeDtypeStruct((NCORES * a.shape[0], *a.shape[1:]), a.dtype)
             for a in out_avals]
    dsc = jax.export.DisabledSafetyCheck.custom_call("bass_exec")
    return jax.export.export(sharded, disabled_checks=[dsc])(*args, *zargs)


def _get_exported():
    import jax.export
    _install_neff_disk_cache()
    _patch_bass_effect()
    path = _export_cache_path()
    if os.path.exists(path):
        try:
            return jax.export.deserialize(open(path, "rb").read())
        except Exception:
            pass
    exp = _make_exported()
    try:
        os.makedirs(NEFF_CACHE_DIR, exist_ok=True)
        tmp = path + ".tmp"
        with open(tmp, "wb") as f:
            f.write(exp.serialize())
        os.replace(tmp, path)
    except Exception:
        pass
    return exp


def _kernel_bass(x, consts):
    global LAST
    import jax
    exp = _get_exported()
    import concourse.bass2jax as b2j
    b2j.install_neuronx_cc_hook()

    xs = np.concatenate([
        np.ascontiguousarray(x[i // 4, :, (i % 4) * 64:(i % 4) * 64 + 64, :])
        .reshape(2, 128, HSH, 256).astype(BF16)
        for i in range(NCORES)], axis=0)
    gin = [xs] + [np.concatenate([consts[nm]] * NCORES, axis=0)
                  for nm in IN_NAMES[1:]]
    import jax.numpy as jnp
    from jax.sharding import Mesh, PartitionSpec, NamedSharding
    mesh = Mesh(np.asarray(jax.devices()[:NCORES]), ("core",))
    sh = NamedSharding(mesh, PartitionSpec("core"))
    zfn = jax.jit(lambda: jnp.zeros((NCORES * 2, 128, HSH, 256), BF16),
                  out_shardings=sh)
    z = zfn()
    out = jax.jit(exp.call, in_shardings=(sh,) * (len(gin) + 1),
                  out_shardings=sh)(*gin, z)
    out0 = np.asarray(out[0] if isinstance(out, (tuple, list)) else out)
    res = out0.reshape(NCORES, 2, 128, HSH, 256)
    full = np.empty((2, DIM, 256, 256), dtype=np.float32)
    for i in range(NCORES):
        b, r0 = i // 4, (i % 4) * 64
        full[b, :, r0:r0 + 64, :] = res[i].reshape(256, 64, 256).astype(np.float32)
    return full


def _kernel_numpy(x, norm_g, norm_b, w_qkv, w_out, b_out, **dpb):
    # fallback: straight port of the reference in numpy (f32)
    B, D, H, W = x.shape
    nh, nw = H // WSZ, W // WSZ
    mean = x.mean(axis=1, keepdims=True)
    var = x.var(axis=1, keepdims=True)
    xn = (x - mean) / np.sqrt(var + EPS) * norm_g + norm_b
    xw = xn.reshape(B, D, nh, WSZ, nw, WSZ).transpose(0, 2, 4, 1, 3, 5)
    xw = xw.reshape(B * nh * nw, D, WSZ * WSZ)
    qkv = np.einsum('ed,bdn->ben', w_qkv, xw)
    q, k, v = np.split(qkv, 3, axis=1)
    th = lambda t: t.reshape(-1, HEADS, DIM_HEAD, WSZ * WSZ).transpose(0, 1, 3, 2)
    q, k, v = th(q) * SCALE, th(k), th(v)
    sim = np.einsum('bhid,bhjd->bhij', q, k)
    sim = sim + _dpb_bias64(**dpb)[None, None]
    sim = sim - sim.max(-1, keepdims=True)
    e = np.exp(sim)
    attn = e / e.sum(-1, keepdims=True)
    o = np.einsum('bhij,bhjd->bhid', attn, v)
    o = o.transpose(0, 1, 3, 2).reshape(-1, HEADS * DIM_HEAD, WSZ * WSZ)
    o = np.einsum('de,ben->bdn', w_out, o) + b_out[None, :, None]
    o = o.reshape(B, nh, nw, D, WSZ, WSZ).transpose(0, 3, 1, 4, 2, 5).reshape(B, D, H, W)
    return o.astype(np.float32)


def kernel(**inputs):
    inputs = {k: np.asarray(v) for k, v in inputs.items()}
    try:
        consts = prep_consts(**{k: v for k, v in inputs.items() if k != "x"})
        return _kernel_bass(np.asarray(inputs["x"], np.float32), consts)
    except Exception as ex:
        sys.stderr.write(f"[kernel] bass path failed ({ex!r}); numpy fallback\n")
        return _kernel_numpy(**inputs)
